# revision 1
# baseline (speedup 1.0000x reference)
"""Bass/Tile kernel for nn_CustomLSTM: per-frame CNN encode sharded across 8
NeuronCores (round-robin over frames), feats AllGather, then the sequential
softmax-recurrence (the LSTM state is dead code w.r.t. the output) replicated
on every core.

Key structure notes:
- conv1 (3->64, 3x3 s2 p1) runs as a single K=54 matmul per 512-column chunk:
  im2col rows = (frame in pair, ic, ky, kx), built by column-phase compaction
  on DVE (stride-2 cols -> contiguous) + one small SBUF->SBUF DMA per tap
  (partition->free reshape). Two frames share the PE stream via block-diagonal
  weights (M = 2x64 oc).
- conv2 (64->128, 3x3 s2 p1) runs per frame-half as 9 accumulating K=64
  matmuls per 512-column chunk against a zero-padded 130x130 image whose taps
  are plain strided APs. relu+bias+mean-pool fuse into one ACT/DVE op with
  accum_out.
- The recurrence keeps u_t = exp(logits) UNNORMALIZED; A'' = fca_w +
  outer(fca_b, 1) makes A''.u = s*(A.o + b), relu scales, and 1/s is applied
  as the dynamic ACT scale of the next exp. Outputs are normalized at the end
  from R[t] = 1/s_t.
"""
import sys

sys.path.insert(0, "/opt/trn_rl_repo")

import numpy as np
import ml_dtypes

import concourse.bass as bass
import concourse.bacc as bacc
import concourse.mybir as mybir
import concourse.tile as tile

BF16 = ml_dtypes.bfloat16
F32 = mybir.dt.float32
BF = mybir.dt.bfloat16
AF = mybir.ActivationFunctionType
ALU = mybir.AluOpType


def build_program(ncores=8, npairs=8, nsteps=128):
    nfr = 2 * npairs                 # frames per core
    T = ncores * nfr                 # total frames/steps available
    assert nsteps <= T
    nc = bacc.Bacc("TRN2", target_bir_lowering=False, debug=False,
                   num_devices=ncores)

    din = lambda n, s, d: nc.dram_tensor(n, s, d, kind="ExternalInput")
    frames = din("frames", [nfr, 3, 256, 256], F32)
    w1bd = din("w1bd", [54, 128], BF)
    b1 = din("b1", [128, 1], F32)
    w2 = din("w2", [128, 9, 128], BF)
    b2 = din("b2", [128, 1], F32)
    enc = din("enc", [14, 2048], F32)
    fceT = din("fceT", [128, 2, 16, 128], BF)
    fceb2 = din("fceb2", [128, 1], F32)
    fcw2T = din("fcw2T", [128, 7, 400], BF)
    fcw1T = din("fcw1T", [128, 400], BF)
    fcb = din("fcb", [100, 4], F32)
    AT = din("AT", [100, 4, 512], BF)
    A2T = din("A2T", [100, 4, 512], BF)
    baT = din("baT", [128, 4], F32)
    F3T = din("F3T", [128, 4, 400], BF)
    act_in = din("act_in", [1, 400], F32)
    ident = din("ident", [128, 128], F32)
    identb = din("identb", [128, 128], BF)

    out = nc.dram_tensor("out", [nsteps, 400], F32, kind="ExternalOutput")
    feats_loc = nc.dram_tensor("feats_loc", [128, nfr], F32)
    feats_glob = nc.dram_tensor("feats_glob", [128 * ncores, nfr], F32,
                                addr_space="Shared")

    with tile.TileContext(nc) as tc:
        with tc.tile_pool(name="const", bufs=1) as cp:
            w1bd_t = cp.tile([54, 128], BF)
            nc.sync.dma_start(out=w1bd_t[:], in_=w1bd[:])
            b1_t = cp.tile([128, 1], F32)
            nc.sync.dma_start(out=b1_t[:], in_=b1[:])
            w2_t = cp.tile([128, 9, 128], BF)
            nc.sync.dma_start(out=w2_t[:], in_=w2[:])
            b2_t = cp.tile([128, 1], F32)
            nc.sync.dma_start(out=b2_t[:], in_=b2[:])
            ident_t = cp.tile([128, 128], F32)
            nc.sync.dma_start(out=ident_t[:], in_=ident[:])
            identb_t = cp.tile([128, 128], BF)
            nc.sync.dma_start(out=identb_t[:], in_=identb[:])

            # ---------------- conv stage ----------------
            with tc.tile_pool(name="cv", bufs=2) as cv, \
                 tc.tile_pool(name="cv1", bufs=1) as cv1, \
                 tc.tile_pool(name="ps1", bufs=2, space="PSUM") as ps1, \
                 tc.tile_pool(name="ps2", bufs=4, space="PSUM") as ps2:
                feats_sb = cv1.tile([128, nfr], F32)
                for j in range(npairs):
                    FR = cv.tile([128, 2, 3, 2, 128, 2], F32, tag="FR")
                    for f in range(2):
                        nc.sync.dma_start(out=FR[:, f], in_=frames[2 * j + f].rearrange("c (p r) x -> p c (r x)", r=2))
                    # column-phase compaction (cast f32 -> bf16)
                    Q = {}
                    for f in range(2):
                        for rp in range(2):  # row parity
                            for cpar in range(2):  # col parity
                                q = cv.tile([128, 3, 130], BF,
                                            tag=f"Q{f}{rp}{cpar}")
                                Q[(f, rp, cpar)] = q
                                j0 = cpar  # odd cols shift right by one
                                nc.vector.tensor_copy(
                                    q[:, :, j0:j0 + 128],
                                    FR[:, f, :, rp, :, cpar])
                                if cpar == 1:
                                    nc.vector.memset(q[:, :, 0:1], 0.0)
                    IC = cv.tile([54, 128, 128], BF, tag="IC")
                    nc.vector.memset(IC[:, 0:1, :], 0.0)
                    for f in range(2):
                        for ic in range(3):
                            for ky in range(3):
                                for kx in range(3):
                                    t = 27 * f + 9 * ic + 3 * ky + kx
                                    rp = (ky + 1) % 2  # ky=1 -> even rows
                                    cpar = (kx + 1) % 2
                                    q = Q[(f, rp, cpar)]
                                    j0 = 0 if kx < 2 else 1
                                    if ky == 0:
                                        src = q[0:127, ic, j0:j0 + 128]
                                        dst = IC[t:t + 1, 1:128, :]
                                    else:
                                        p0 = 0 if ky == 1 else 0
                                        src = q[0:128, ic, j0:j0 + 128]
                                        dst = IC[t:t + 1, 0:128, :]
                                    nc.sync.dma_start(out=dst, in_=src)
                    ICf = IC.rearrange("t a b -> t (a b)")
                    c2p = cv1.tile([128, 130, 130], BF, tag="c2p")
                    nc.vector.memset(c2p[:, 0, :], 0.0)
                    nc.vector.memset(c2p[:, 129, :], 0.0)
                    nc.vector.memset(c2p[:, :, 0:1], 0.0)
                    nc.vector.memset(c2p[:, :, 129:130], 0.0)
                    for n in range(32):
                        pm = ps1.tile([128, 512], F32, tag="pm")
                        nc.tensor.matmul(pm[:], w1bd_t[:],
                                         ICf[:, 512 * n:512 * n + 512],
                                         start=True, stop=True)
                        dst = c2p[:, 1 + 4 * n:5 + 4 * n, 1:129]
                        src = pm.rearrange("p (a b) -> p a b", b=128)
                        if n % 2 == 0:
                            nc.scalar.activation(dst, src, AF.Relu,
                                                 bias=b1_t[:, 0:1])
                        else:
                            nc.vector.tensor_scalar(
                                out=dst, in0=src, scalar1=b1_t[:, 0:1],
                                scalar2=0.0, op0=ALU.add, op1=ALU.max)
                    # conv2 per frame half
                    facc = cv.tile([128, 2, 8], F32, tag="facc")
                    for f in range(2):
                        for n2 in range(8):
                            pc = ps2.tile([128, 512], F32, tag="pc")
                            for i, (ky, kx) in enumerate(
                                    [(a, b) for a in range(3)
                                     for b in range(3)]):
                                rhs = c2p[64 * f:64 * f + 64,
                                          ky + 16 * n2:ky + 16 * n2 + 16:2,
                                          kx:kx + 128:2]
                                nc.tensor.matmul(pc[:], w2_t[64*f:64*f+64, 3 * ky + kx, :],
                                                 rhs, start=(i == 0),
                                                 stop=(i == 8))
                            junk = cv.tile([128, 512], BF, tag="junk")
                            if n2 % 2 == 0:
                                nc.scalar.activation(
                                    junk[:], pc[:], AF.Relu,
                                    bias=b2_t[:, 0:1],
                                    accum_out=facc[:, f, n2:n2 + 1])
                            else:
                                nc.vector.tensor_scalar(
                                    out=junk[:], in0=pc[:],
                                    scalar1=b2_t[:, 0:1], scalar2=0.0,
                                    op0=ALU.add, op1=ALU.max,
                                    accum_out=facc[:, f, n2:n2 + 1])
                    for f in range(2):
                        nc.vector.reduce_sum(
                            out=feats_sb[:, 2 * j + f:2 * j + f + 1],
                            in_=facc[:, f, :], axis=mybir.AxisListType.X)
                nc.vector.tensor_scalar_mul(feats_sb[:], feats_sb[:],
                                            1.0 / 4096.0)
                nc.sync.dma_start(out=feats_loc[:], in_=feats_sb[:])
                nc.gpsimd.collective_compute(
                    "AllGather", ALU.bypass,
                    replica_groups=[list(range(ncores))],
                    ins=[feats_loc[:]], outs=[feats_glob[:]])

            # ---------------- ev branch + Lpre ----------------
            sc1 = tc.tile_pool(name="sc1", bufs=1)
            scp = sc1.__enter__()
            Lpre = scp.tile([100, 4, nsteps], F32)
            U = scp.tile([100, 4, nsteps], BF)
            R = scp.tile([1, nsteps], F32)
            ones_bf = scp.tile([128, 1], BF)
            nc.vector.memset(ones_bf[:], 1.0)
            ones_row = scp.tile([1, 128], F32)
            nc.vector.memset(ones_row[:], 1.0)

            with tc.tile_pool(name="ev", bufs=2) as evp, \
                 tc.tile_pool(name="pse", bufs=2, space="PSUM") as pse, \
                 tc.tile_pool(name="pse1", bufs=1, space="PSUM") as pse1:
                # gather feats -> [128, T] bf16, t = ncores*jloc + c
                fa = evp.tile([128, ncores, nfr], F32)
                nc.sync.dma_start(out=fa[:], in_=feats_glob[:].rearrange(
                    "(c p) j -> p c j", p=128))
                fb = scp.tile([128, T], BF)
                fbv = fb.rearrange("p (j c) -> p c j", c=ncores)
                nc.vector.tensor_copy(fbv, fa[:])

                # encoded_video branch
                enc_sb = evp.tile([14, 2048], F32)
                nc.sync.dma_start(out=enc_sb[:], in_=enc[:])
                fceT_t = evp.tile([128, 2, 16, 128], BF, tag="fceT")
                nc.sync.dma_start(out=fceT_t[:], in_=fceT[:])
                encT = evp.tile([128, 16, 14], BF)
                for k in range(16):
                    pt = pse.tile([128, 14], F32, tag="pt")
                    nc.tensor.transpose(pt[:], enc_sb[:, 128 * k:128 * k + 128],
                                        ident_t[0:14, 0:14])
                    nc.vector.tensor_copy(encT[:, k, :], pt[:])
                pev = pse1.tile([128, 7], F32)
                for k in range(16):
                    for par in range(2):
                        nc.tensor.matmul(
                            pev[:], fceT_t[:, par, k, :],
                            encT[:, k, par:14:2],
                            start=(k == 0 and par == 0),
                            stop=(k == 15 and par == 1))
                fceb2_t = evp.tile([128, 1], F32)
                nc.sync.dma_start(out=fceb2_t[:], in_=fceb2[:])
                ev_sb = evp.tile([128, 7], BF)
                nc.scalar.activation(ev_sb[:], pev[:], AF.Relu,
                                     bias=fceb2_t[:, 0:1])
                fcw2T_t = evp.tile([128, 7, 400], BF, tag="fcw2T")
                nc.sync.dma_start(out=fcw2T_t[:], in_=fcw2T[:])
                pevl = pse1.tile([100, 4], F32)
                for m in range(4):
                    for k in range(7):
                        nc.tensor.matmul(
                            pevl[:, m:m + 1],
                            fcw2T_t[:, k, 100 * m:100 * m + 100],
                            ev_sb[:, k:k + 1], start=(k == 0), stop=(k == 6))
                fcb_t = evp.tile([100, 4], F32)
                nc.sync.dma_start(out=fcb_t[:], in_=fcb[:])
                bconst = evp.tile([100, 4], F32)
                nc.vector.tensor_add(bconst[:], pevl[:], fcb_t[:])
                fcw1T_t = evp.tile([128, 400], BF)
                nc.sync.dma_start(out=fcw1T_t[:], in_=fcw1T[:])
                for m in range(4):
                    plp = pse.tile([100, nsteps], F32, tag="plp")
                    nc.tensor.matmul(plp[:],
                                     fcw1T_t[:, 100 * m:100 * m + 100],
                                     fb[:, 0:nsteps], start=True, stop=True)
                    nc.vector.tensor_scalar_add(Lpre[:, m, :], plp[:],
                                                bconst[:, m:m + 1])

            # ---------------- scan ----------------
            AT_t = scp.tile([100, 4, 512], BF)
            nc.sync.dma_start(out=AT_t[:], in_=AT[:])
            A2T_t = scp.tile([100, 4, 512], BF)
            nc.sync.dma_start(out=A2T_t[:], in_=A2T[:])
            baT_t = scp.tile([128, 4], F32)
            nc.sync.dma_start(out=baT_t[:], in_=baT[:])
            F3T_t = scp.tile([128, 4, 400], BF)
            nc.sync.dma_start(out=F3T_t[:], in_=F3T[:])

            with tc.tile_pool(name="sstep", bufs=3) as ssp, \
                 tc.tile_pool(name="psA", bufs=2, space="PSUM") as psA, \
                 tc.tile_pool(name="psL", bufs=2, space="PSUM") as psL, \
                 tc.tile_pool(name="psS", bufs=2, space="PSUM") as psS:
                # step 0 input: transpose action row into [100, 4]
                act_row = ssp.tile([1, 400], F32, tag="actrow")
                nc.sync.dma_start(out=act_row[:], in_=act_in[:])
                u0 = ssp.tile([100, 4], BF, tag="u0")
                for m in range(4):
                    pa0 = psS.tile([100, 1], F32, tag="small")
                    nc.tensor.transpose(pa0[:],
                                        act_row[0:1, 100 * m:100 * m + 100],
                                        ident_t[0:1, 0:1])
                    nc.vector.tensor_copy(u0[:, m:m + 1], pa0[:])

                r_prev = None
                for t in range(nsteps):
                    Wmat = AT_t if t == 0 else A2T_t
                    ucur = (lambda k: u0[:, k:k + 1]) if t == 0 else \
                        (lambda k, tt=t - 1: U[0:100, k, tt:tt + 1])
                    a_ps = psA.tile([128, 4], F32, tag="a_ps")
                    for m in range(4):
                        for k in range(4):
                            nc.tensor.matmul(
                                a_ps[:, m:m + 1],
                                Wmat[:, k, 128 * m:128 * m + 128],
                                ucur(k), start=(k == 0), stop=(k == 3))
                    w_sb = ssp.tile([128, 4], BF, tag="w_sb")
                    if t == 0:
                        for m in range(4):
                            nc.vector.tensor_scalar(
                                out=w_sb[:, m:m + 1], in0=a_ps[:, m:m + 1],
                                scalar1=baT_t[:, m:m + 1], scalar2=0.0,
                                op0=ALU.add, op1=ALU.max)
                    else:
                        nc.vector.tensor_scalar_max(w_sb[:], a_ps[:], 0.0)
                    l_ps = psL.tile([100, 4], F32, tag="l_ps")
                    for m in range(4):
                        for k in range(4):
                            nc.tensor.matmul(
                                l_ps[:, m:m + 1],
                                F3T_t[:, k, 100 * m:100 * m + 100],
                                w_sb[:, k:k + 1], start=(k == 0), stop=(k == 3))
                    for m in range(4):
                        nc.scalar.activation(
                            U[0:100, m, t:t + 1], l_ps[0:100, m:m + 1],
                            AF.Exp, bias=Lpre[0:100, m, t:t + 1],
                            scale=(1.0 if t == 0 else r_prev[0:100, 0:1]))
                    # side chain: R[t] = 1/sum(u_t); broadcast for next exp
                    s_ps = psS.tile([1, 4], F32, tag="small")
                    nc.tensor.matmul(s_ps[:], ones_bf[0:100, 0:1],
                                     U[0:100, :, t], start=True, stop=True)
                    s_sb = ssp.tile([1, 1], F32, tag="s_sb")
                    nc.vector.reduce_sum(out=s_sb[:], in_=s_ps[:],
                                         axis=mybir.AxisListType.X)
                    nc.vector.reciprocal(R[0:1, t:t + 1], s_sb[:])
                    if t + 1 < nsteps:
                        rb_ps = psS.tile([128, 1], F32, tag="small")
                        nc.tensor.matmul(rb_ps[:], ones_row[:],
                                         R[0:1, t:t + 1], start=True,
                                         stop=True)
                        r_bc = ssp.tile([128, 1], F32, tag="r_bc")
                        nc.vector.tensor_copy(r_bc[:], rb_ps[:])
                        r_prev = r_bc

                # final: transpose U to [t, c] layout and normalize by R
                rt_ps = psS.tile([128, 1], F32, tag="small")
                nc.tensor.transpose(rt_ps[0:nsteps, 0:1], R[:, 0:nsteps],
                                    ident_t[0:1, 0:1])
                rcol = ssp.tile([128, 1], F32, tag="rcol")
                nc.vector.tensor_copy(rcol[0:nsteps], rt_ps[0:nsteps])
                OT = ssp.tile([128, 400], F32, tag="OT")
                for m in range(4):
                    tp_ps = psA.tile([128, 100], BF, tag="tp_ps")
                    nc.tensor.transpose(tp_ps[0:nsteps, :],
                                        U[0:100, m, 0:nsteps],
                                        identb_t[0:100, 0:100])
                    nc.vector.tensor_scalar_mul(
                        OT[0:nsteps, 100 * m:100 * m + 100],
                        tp_ps[0:nsteps, :], rcol[0:nsteps, 0:1])
                nc.sync.dma_start(out=out[:], in_=OT[0:nsteps, :])
            sc1.__exit__(None, None, None)

    nc.compile()
    return nc


def prep_weights(inputs, ncores=8):
    """Host-side numpy prep of all weight layouts. Returns dict of arrays
    shared by all cores (frames excluded)."""
    f32 = np.float32
    conv1_w = np.asarray(inputs["conv1_w"], f32)
    conv2_w = np.asarray(inputs["conv2_w"], f32)
    w1bd = np.zeros((54, 128), f32)
    for f in range(2):
        for ic in range(3):
            for ky in range(3):
                for kx in range(3):
                    t = 27 * f + 9 * ic + 3 * ky + kx
                    w1bd[t, 64 * f:64 * f + 64] = conv1_w[:, ic, ky, kx]
    b1 = np.tile(np.asarray(inputs["conv1_b"], f32), 2).reshape(128, 1)
    w2 = np.transpose(conv2_w, (1, 2, 3, 0)).reshape(64, 9, 128)
    w2 = np.concatenate([w2, w2], axis=0)
    b2 = np.asarray(inputs["conv2_b"], f32).reshape(128, 1)

    enc = np.asarray(inputs["encoded_video"], f32).reshape(14, 2048)
    fce_w = np.asarray(inputs["fce_w"], f32)      # [64, 2048]
    fceT = np.zeros((128, 2, 16, 128), f32)
    fceTf = fce_w.T.reshape(16, 128, 64)      # [k, p, o]
    fceT[:, 0, :, 0:64] = fceTf.transpose(1, 0, 2)
    fceT[:, 1, :, 64:128] = fceTf.transpose(1, 0, 2)
    fceb2 = np.tile(np.asarray(inputs["fce_b"], f32), 2).reshape(128, 1)

    fc_w = np.asarray(inputs["fc_w"], f32)        # [400, 1536]
    fc_b = np.asarray(inputs["fc_b"], f32)
    fcw1T = fc_w[:, 0:128].T.copy()               # [128, 400]
    fcw2T = fc_w[:, 128:1024].T.reshape(7, 128, 400).transpose(1, 0, 2).copy()
    F3T = fc_w[:, 1024:1536].T.reshape(4, 128, 400).transpose(1, 0, 2).copy()
    fcb = fc_b.reshape(4, 100).T.copy()           # [100, 4]

    fca_w = np.asarray(inputs["fca_w"], f32)      # [512, 400]
    fca_b = np.asarray(inputs["fca_b"], f32)
    ATm = fca_w.T.reshape(4, 100, 512).transpose(1, 0, 2).copy()
    A2 = fca_w + fca_b[:, None]
    A2Tm = A2.T.reshape(4, 100, 512).transpose(1, 0, 2).copy()
    baT = fca_b.reshape(4, 128).T.copy()          # [128, 4]

    action = np.asarray(inputs["action"], f32).reshape(1, 400)

    bf = lambda x: np.ascontiguousarray(x).astype(BF16)
    return {
        "w1bd": bf(w1bd), "b1": b1, "w2": bf(w2), "b2": b2,
        "enc": enc, "fceT": bf(fceT), "fceb2": fceb2,
        "fcw2T": bf(fcw2T), "fcw1T": bf(fcw1T), "fcb": fcb,
        "AT": bf(ATm), "A2T": bf(A2Tm), "baT": baT, "F3T": bf(F3T),
        "act_in": action,
        "ident": np.eye(128, dtype=f32),
        "identb": np.eye(128, dtype=f32).astype(BF16),
    }


_CACHE = {}


def _run(inputs, trace=False):
    from concourse.bass_utils import run_bass_kernel_spmd
    ncores = 8
    if "nc" not in _CACHE:
        _CACHE["nc"] = build_program(ncores=ncores, npairs=8, nsteps=128)
    nc = _CACHE["nc"]
    common = prep_weights(inputs, ncores)
    vf = np.asarray(inputs["video_frame"], np.float32)[0]
    in_maps = []
    for c in range(ncores):
        m = dict(common)
        m["frames"] = np.ascontiguousarray(vf[c::ncores])
        in_maps.append(m)
    return run_bass_kernel_spmd(nc, in_maps, core_ids=list(range(ncores)),
                                trace=trace)


def kernel(**inputs):
    res = _run(inputs, trace=False)
    return res.results[0]["out"].reshape(1, 128, 400).astype(np.float32)



# revision 8
# speedup vs baseline: 1.3162x; 1.3162x over previous
"""Bass/Tile kernel for nn_CustomLSTM: per-frame CNN encode sharded across 8
NeuronCores (round-robin over frames), feats AllGather, then the sequential
softmax-recurrence (the LSTM state is dead code w.r.t. the output) replicated
on every core.

Key structure notes:
- conv1 (3->64, 3x3 s2 p1) runs as a single K=54 matmul per 512-column chunk:
  im2col rows = (frame in pair, ic, ky, kx), built by column-phase compaction
  on DVE (stride-2 cols -> contiguous) + one small SBUF->SBUF DMA per tap
  (partition->free reshape). Two frames share the PE stream via block-diagonal
  weights (M = 2x64 oc).
- conv2 (64->128, 3x3 s2 p1) runs per frame-half as 9 accumulating K=64
  matmuls per 512-column chunk against a zero-padded 130x130 image whose taps
  are plain strided APs. relu+bias+mean-pool fuse into one ACT/DVE op with
  accum_out.
- The recurrence keeps u_t = exp(logits) UNNORMALIZED; A'' = fca_w +
  outer(fca_b, 1) makes A''.u = s*(A.o + b), relu scales, and 1/s is applied
  as the dynamic ACT scale of the next exp. Outputs are normalized at the end
  from R[t] = 1/s_t.
"""
import sys

sys.path.insert(0, "/opt/trn_rl_repo")

import numpy as np
import ml_dtypes

import concourse.bass as bass
import concourse.bacc as bacc
import concourse.mybir as mybir
import concourse.tile as tile

BF16 = ml_dtypes.bfloat16
F32 = mybir.dt.float32
BF = mybir.dt.bfloat16
AF = mybir.ActivationFunctionType
ALU = mybir.AluOpType


def build_program(ncores=8, npairs=8, nsteps=128):
    nfr = 2 * npairs                 # frames per core
    T = ncores * nfr                 # total frames/steps available
    assert nsteps <= T
    nc = bacc.Bacc("TRN2", target_bir_lowering=False, debug=False,
                   num_devices=ncores)

    din = lambda n, s, d: nc.dram_tensor(n, s, d, kind="ExternalInput")
    frames = din("frames", [nfr, 3, 256, 256], F32)
    w1bd = din("w1bd", [54, 128], BF)
    b1 = din("b1", [128, 1], F32)
    w2 = din("w2", [128, 9, 128], BF)
    b2 = din("b2", [128, 1], F32)
    enc = din("enc", [14, 2048], F32)
    fceT = din("fceT", [128, 2, 16, 128], BF)
    fceb2 = din("fceb2", [128, 1], F32)
    fcw2T = din("fcw2T", [128, 7, 400], BF)
    fcw1T = din("fcw1T", [128, 400], BF)
    fcb = din("fcb", [100, 4], F32)
    AT = din("AT", [100, 4, 512], BF)
    A2T = din("A2T", [100, 4, 512], BF)
    baT = din("baT", [128, 4], F32)
    F3Tp = din("F3Tp", [128, 4, 4, 128], BF)
    act_in = din("act_in", [1, 400], F32)
    ident = din("ident", [128, 128], F32)
    identb = din("identb", [128, 128], BF)

    out = nc.dram_tensor("out", [nsteps, 400], F32, kind="ExternalOutput")
    feats_loc = nc.dram_tensor("feats_loc", [128, nfr], F32)
    feats_glob = nc.dram_tensor("feats_glob", [128 * ncores, nfr], F32,
                                addr_space="Shared")

    with tile.TileContext(nc) as tc:
        with tc.tile_pool(name="const", bufs=1) as cp:
            w1bd_t = cp.tile([54, 128], BF)
            nc.sync.dma_start(out=w1bd_t[:], in_=w1bd[:])
            b1_t = cp.tile([128, 1], F32)
            nc.sync.dma_start(out=b1_t[:], in_=b1[:])
            w2_t = cp.tile([128, 9, 128], BF)
            nc.sync.dma_start(out=w2_t[:], in_=w2[:])
            b2_t = cp.tile([128, 1], F32)
            nc.sync.dma_start(out=b2_t[:], in_=b2[:])
            ident_t = cp.tile([128, 128], F32)
            nc.sync.dma_start(out=ident_t[:], in_=ident[:])
            identb_t = cp.tile([128, 128], BF)
            nc.sync.dma_start(out=identb_t[:], in_=identb[:])

            # ---------------- conv stage ----------------
            with tc.tile_pool(name="cv", bufs=2) as cv, \
                 tc.tile_pool(name="cv1", bufs=1) as cv1, \
                 tc.tile_pool(name="ps1", bufs=2, space="PSUM") as ps1, \
                 tc.tile_pool(name="ps2", bufs=4, space="PSUM") as ps2:
                feats_sb = cv1.tile([128, nfr], F32)
                for j in range(npairs):
                    FR = cv.tile([128, 2, 3, 2, 128, 2], F32, tag="FR")
                    for f in range(2):
                        nc.sync.dma_start(out=FR[:, f], in_=frames[2 * j + f].rearrange("c (p r) x -> p c (r x)", r=2))
                    # column-phase compaction (cast f32 -> bf16)
                    Q = {}
                    for f in range(2):
                        for rp in range(2):  # row parity
                            for cpar in range(2):  # col parity
                                q = cv.tile([128, 3, 130], BF,
                                            tag=f"Q{f}{rp}{cpar}")
                                Q[(f, rp, cpar)] = q
                                j0 = cpar  # odd cols shift right by one
                                nc.vector.tensor_copy(
                                    q[:, :, j0:j0 + 128],
                                    FR[:, f, :, rp, :, cpar])
                                if cpar == 1:
                                    nc.vector.memset(q[:, :, 0:1], 0.0)
                    IC = cv.tile([54, 128, 128], BF, tag="IC")
                    nc.vector.memset(IC[:, 0:1, :], 0.0)
                    for f in range(2):
                        for ic in range(3):
                            for ky in range(3):
                                for kx in range(3):
                                    t = 27 * f + 9 * ic + 3 * ky + kx
                                    rp = (ky + 1) % 2  # ky=1 -> even rows
                                    cpar = (kx + 1) % 2
                                    q = Q[(f, rp, cpar)]
                                    j0 = 0 if kx < 2 else 1
                                    if ky == 0:
                                        src = q[0:127, ic, j0:j0 + 128]
                                        dst = IC[t:t + 1, 1:128, :]
                                    else:
                                        p0 = 0 if ky == 1 else 0
                                        src = q[0:128, ic, j0:j0 + 128]
                                        dst = IC[t:t + 1, 0:128, :]
                                    nc.sync.dma_start(out=dst, in_=src)
                    ICf = IC.rearrange("t a b -> t (a b)")
                    c2p = cv1.tile([128, 130, 130], BF, tag="c2p")
                    nc.vector.memset(c2p[:, 0, :], 0.0)
                    nc.vector.memset(c2p[:, 129, :], 0.0)
                    nc.vector.memset(c2p[:, :, 0:1], 0.0)
                    nc.vector.memset(c2p[:, :, 129:130], 0.0)
                    for n in range(32):
                        pm = ps1.tile([128, 512], F32, tag="pm")
                        nc.tensor.matmul(pm[:], w1bd_t[:],
                                         ICf[:, 512 * n:512 * n + 512],
                                         start=True, stop=True)
                        dst = c2p[:, 1 + 4 * n:5 + 4 * n, 1:129]
                        src = pm.rearrange("p (a b) -> p a b", b=128)
                        if n % 2 == 0:
                            nc.scalar.activation(dst, src, AF.Relu,
                                                 bias=b1_t[:, 0:1])
                        else:
                            nc.vector.tensor_scalar(
                                out=dst, in0=src, scalar1=b1_t[:, 0:1],
                                scalar2=0.0, op0=ALU.add, op1=ALU.max)
                    # conv2 per frame half
                    facc = cv.tile([128, 2, 8], F32, tag="facc")
                    for f in range(2):
                        for n2 in range(8):
                            pc = ps2.tile([128, 512], F32, tag="pc")
                            for i, (ky, kx) in enumerate(
                                    [(a, b) for a in range(3)
                                     for b in range(3)]):
                                rhs = c2p[64 * f:64 * f + 64,
                                          ky + 16 * n2:ky + 16 * n2 + 16:2,
                                          kx:kx + 128:2]
                                nc.tensor.matmul(pc[:], w2_t[64*f:64*f+64, 3 * ky + kx, :],
                                                 rhs, start=(i == 0),
                                                 stop=(i == 8))
                            junk = cv.tile([128, 512], BF, tag="junk")
                            if n2 % 2 == 0:
                                nc.scalar.activation(
                                    junk[:], pc[:], AF.Relu,
                                    bias=b2_t[:, 0:1],
                                    accum_out=facc[:, f, n2:n2 + 1])
                            else:
                                nc.vector.tensor_scalar(
                                    out=junk[:], in0=pc[:],
                                    scalar1=b2_t[:, 0:1], scalar2=0.0,
                                    op0=ALU.add, op1=ALU.max,
                                    accum_out=facc[:, f, n2:n2 + 1])
                    for f in range(2):
                        nc.vector.reduce_sum(
                            out=feats_sb[:, 2 * j + f:2 * j + f + 1],
                            in_=facc[:, f, :], axis=mybir.AxisListType.X)
                nc.vector.tensor_scalar_mul(feats_sb[:], feats_sb[:],
                                            1.0 / 4096.0)
                nc.sync.dma_start(out=feats_loc[:], in_=feats_sb[:])
                nc.gpsimd.collective_compute(
                    "AllGather", ALU.bypass,
                    replica_groups=[list(range(ncores))],
                    ins=[feats_loc[:]], outs=[feats_glob[:]])

            # ---------------- ev branch + Lpre ----------------
            sc1 = tc.tile_pool(name="sc1", bufs=1)
            scp = sc1.__enter__()
            Lpre = scp.tile([100, 4, nsteps], F32)
            Lh = scp.tile([100, 4, nsteps], BF)
            Ll = scp.tile([100, 4, nsteps], BF)
            U = scp.tile([100, 4, nsteps], BF)
            ones100 = scp.tile([100, 128], BF)
            nc.vector.memset(ones100[:], 1.0)

            with tc.tile_pool(name="ev", bufs=2) as evp, \
                 tc.tile_pool(name="pse", bufs=2, space="PSUM") as pse, \
                 tc.tile_pool(name="pse1", bufs=1, space="PSUM") as pse1:
                # gather feats -> [128, T] bf16, t = ncores*jloc + c
                fa = evp.tile([128, ncores, nfr], F32)
                nc.sync.dma_start(out=fa[:], in_=feats_glob[:].rearrange(
                    "(c p) j -> p c j", p=128))
                fb = scp.tile([128, T], BF)
                fbv = fb.rearrange("p (j c) -> p c j", c=ncores)
                nc.vector.tensor_copy(fbv, fa[:])

                # encoded_video branch
                enc_sb = evp.tile([14, 2048], F32)
                nc.sync.dma_start(out=enc_sb[:], in_=enc[:])
                fceT_t = evp.tile([128, 2, 16, 128], BF, tag="fceT")
                nc.sync.dma_start(out=fceT_t[:], in_=fceT[:])
                encT = evp.tile([128, 16, 14], BF)
                for k in range(16):
                    pt = pse.tile([128, 14], F32, tag="pt")
                    nc.tensor.transpose(pt[:], enc_sb[:, 128 * k:128 * k + 128],
                                        ident_t[0:14, 0:14])
                    nc.vector.tensor_copy(encT[:, k, :], pt[:])
                pev = pse1.tile([128, 7], F32)
                for k in range(16):
                    for par in range(2):
                        nc.tensor.matmul(
                            pev[:], fceT_t[:, par, k, :],
                            encT[:, k, par:14:2],
                            start=(k == 0 and par == 0),
                            stop=(k == 15 and par == 1))
                fceb2_t = evp.tile([128, 1], F32)
                nc.sync.dma_start(out=fceb2_t[:], in_=fceb2[:])
                ev_sb = evp.tile([128, 7], BF)
                nc.scalar.activation(ev_sb[:], pev[:], AF.Relu,
                                     bias=fceb2_t[:, 0:1])
                fcw2T_t = evp.tile([128, 7, 400], BF, tag="fcw2T")
                nc.sync.dma_start(out=fcw2T_t[:], in_=fcw2T[:])
                pevl = pse1.tile([100, 4], F32)
                for m in range(4):
                    for k in range(7):
                        nc.tensor.matmul(
                            pevl[:, m:m + 1],
                            fcw2T_t[:, k, 100 * m:100 * m + 100],
                            ev_sb[:, k:k + 1], start=(k == 0), stop=(k == 6))
                fcb_t = evp.tile([100, 4], F32)
                nc.sync.dma_start(out=fcb_t[:], in_=fcb[:])
                bconst = evp.tile([100, 4], F32)
                nc.vector.tensor_add(bconst[:], pevl[:], fcb_t[:])
                fcw1T_t = evp.tile([128, 400], BF)
                nc.sync.dma_start(out=fcw1T_t[:], in_=fcw1T[:])
                for m in range(4):
                    plp = pse.tile([100, nsteps], F32, tag="plp")
                    nc.tensor.matmul(plp[:],
                                     fcw1T_t[:, 100 * m:100 * m + 100],
                                     fb[:, 0:nsteps], start=True, stop=True)
                    nc.vector.tensor_scalar_add(Lpre[:, m, :], plp[:],
                                                bconst[:, m:m + 1])
                    nc.vector.tensor_copy(Lh[0:100, m, :], Lpre[0:100, m, :])
                    nc.vector.tensor_sub(Ll[0:100, m, :], Lpre[0:100, m, :],
                                         Lh[0:100, m, :])

            # ---------------- scan ----------------
            AT_t = scp.tile([100, 4, 512], BF)
            nc.sync.dma_start(out=AT_t[:], in_=AT[:])
            A2T_t = scp.tile([100, 4, 512], BF)
            nc.sync.dma_start(out=A2T_t[:], in_=A2T[:])
            baT_t = scp.tile([128, 4], F32)
            nc.sync.dma_start(out=baT_t[:], in_=baT[:])
            F3Tp_t = scp.tile([128, 4, 4, 128], BF)
            nc.sync.dma_start(out=F3Tp_t[:], in_=F3Tp[:])

            with tc.tile_pool(name="sstep", bufs=3) as ssp, \
                 tc.tile_pool(name="psA", bufs=2, space="PSUM") as psA, \
                 tc.tile_pool(name="psL", bufs=2, space="PSUM") as psL, \
                 tc.tile_pool(name="psS", bufs=2, space="PSUM") as psS:
                # step 0 input: transpose action row into [100, 4]
                act_row = ssp.tile([1, 400], F32, tag="actrow")
                nc.sync.dma_start(out=act_row[:], in_=act_in[:])
                u0 = ssp.tile([100, 4], BF, tag="u0")
                for m in range(4):
                    pa0 = psS.tile([128, 4], F32, tag="S")
                    nc.tensor.transpose(pa0[0:100, 0:1],
                                        act_row[0:1, 100 * m:100 * m + 100],
                                        ident_t[0:1, 0:1])
                    nc.vector.tensor_copy(u0[:, m:m + 1], pa0[0:100, 0:1])

                for t in range(nsteps):
                    Wmat = AT_t if t == 0 else A2T_t
                    ucur = (lambda k: u0[:, k:k + 1]) if t == 0 else \
                        (lambda k, tt=t - 1: U[0:100, k, tt:tt + 1])
                    # normalizer: S = colsum(u_{t-1}) broadcast to 128 rows
                    if t > 0:
                        S_ps = psS.tile([128, 4], F32, tag="S")
                        nc.tensor.matmul(S_ps[:], ones100[:],
                                         U[0:100, :, t - 1],
                                         start=True, stop=True)
                    a_ps = psA.tile([128, 4], F32, tag="a_ps")
                    for m in range(4):
                        for k in range(4):
                            nc.tensor.matmul(
                                a_ps[:, m:m + 1],
                                Wmat[:, k, 128 * m:128 * m + 128],
                                ucur(k), start=(k == 0), stop=(k == 3))
                    w_sb = ssp.tile([128, 4], BF, tag="w_sb")
                    if t == 0:
                        for m in range(4):
                            nc.vector.tensor_scalar(
                                out=w_sb[:, m:m + 1], in0=a_ps[:, m:m + 1],
                                scalar1=baT_t[:, m:m + 1], scalar2=0.0,
                                op0=ALU.add, op1=ALU.max)
                    else:
                        s1 = ssp.tile([128, 1], F32, tag="s1")
                        nc.vector.reduce_sum(out=s1[:], in_=S_ps[:],
                                             axis=mybir.AxisListType.X)
                        r_t = ssp.tile([128, 1], F32, tag="r_t")
                        nc.vector.reciprocal(r_t[:], s1[:])
                        nc.vector.tensor_scalar(
                            out=w_sb[:], in0=a_ps[:],
                            scalar1=r_t[:, 0:1], scalar2=0.0,
                            op0=ALU.mult, op1=ALU.max)
                    # logits = F3.w + Lpre (hi+lo), all in one PSUM group
                    l_ps = psL.tile([128, 4], F32, tag="l_ps")
                    nc.tensor.matmul(l_ps[:], identb_t[0:100, :],
                                     Lh[0:100, :, t], start=True, stop=False)
                    nc.tensor.matmul(l_ps[:], identb_t[0:100, :],
                                     Ll[0:100, :, t], start=False, stop=False)
                    for m in range(4):
                        for k in range(4):
                            nc.tensor.matmul(
                                l_ps[:, m:m + 1],
                                F3Tp_t[:, k, m, :],
                                w_sb[:, k:k + 1], start=False,
                                stop=(k == 3))
                    nc.scalar.activation(U[0:100, :, t], l_ps[0:100, :],
                                         AF.Exp)

                # final: transpose U to [t, c] layout and normalize rows
                OT = ssp.tile([128, 400], F32, tag="OT")
                for m in range(4):
                    tp_ps = psA.tile([128, 100], BF, tag="tp_ps")
                    nc.tensor.transpose(tp_ps[0:nsteps, :],
                                        U[0:100, m, 0:nsteps],
                                        identb_t[0:100, 0:100])
                    nc.vector.tensor_copy(
                        OT[0:nsteps, 100 * m:100 * m + 100],
                        tp_ps[0:nsteps, :])
                rowsum = ssp.tile([128, 1], F32, tag="rowsum")
                nc.vector.reduce_sum(out=rowsum[0:nsteps],
                                     in_=OT[0:nsteps, :],
                                     axis=mybir.AxisListType.X)
                rinv = ssp.tile([128, 1], F32, tag="rinv")
                nc.vector.reciprocal(rinv[0:nsteps], rowsum[0:nsteps])
                OTn = ssp.tile([128, 400], F32, tag="OTn")
                nc.vector.tensor_scalar_mul(OTn[0:nsteps, :],
                                            OT[0:nsteps, :],
                                            rinv[0:nsteps, 0:1])
                nc.sync.dma_start(out=out[:], in_=OTn[0:nsteps, :])
            sc1.__exit__(None, None, None)

    nc.compile()
    return nc


def prep_weights(inputs, ncores=8):
    """Host-side numpy prep of all weight layouts. Returns dict of arrays
    shared by all cores (frames excluded)."""
    f32 = np.float32
    conv1_w = np.asarray(inputs["conv1_w"], f32)
    conv2_w = np.asarray(inputs["conv2_w"], f32)
    w1bd = np.zeros((54, 128), f32)
    for f in range(2):
        for ic in range(3):
            for ky in range(3):
                for kx in range(3):
                    t = 27 * f + 9 * ic + 3 * ky + kx
                    w1bd[t, 64 * f:64 * f + 64] = conv1_w[:, ic, ky, kx]
    b1 = np.tile(np.asarray(inputs["conv1_b"], f32), 2).reshape(128, 1)
    w2 = np.transpose(conv2_w, (1, 2, 3, 0)).reshape(64, 9, 128)
    w2 = np.concatenate([w2, w2], axis=0)
    b2 = np.asarray(inputs["conv2_b"], f32).reshape(128, 1)

    enc = np.asarray(inputs["encoded_video"], f32).reshape(14, 2048)
    fce_w = np.asarray(inputs["fce_w"], f32)      # [64, 2048]
    fceT = np.zeros((128, 2, 16, 128), f32)
    fceTf = fce_w.T.reshape(16, 128, 64)      # [k, p, o]
    fceT[:, 0, :, 0:64] = fceTf.transpose(1, 0, 2)
    fceT[:, 1, :, 64:128] = fceTf.transpose(1, 0, 2)
    fceb2 = np.tile(np.asarray(inputs["fce_b"], f32), 2).reshape(128, 1)

    fc_w = np.asarray(inputs["fc_w"], f32)        # [400, 1536]
    fc_b = np.asarray(inputs["fc_b"], f32)
    fcw1T = fc_w[:, 0:128].T.copy()               # [128, 400]
    fcw2T = fc_w[:, 128:1024].T.reshape(7, 128, 400).transpose(1, 0, 2).copy()
    F3t = fc_w[:, 1024:1536].T                    # [512, 400]
    F3Tp = np.zeros((128, 4, 4, 128), f32)
    for k in range(4):
        for m in range(4):
            F3Tp[:, k, m, 0:100] = F3t[128 * k:128 * k + 128,
                                       100 * m:100 * m + 100]
    fcb = fc_b.reshape(4, 100).T.copy()           # [100, 4]

    fca_w = np.asarray(inputs["fca_w"], f32)      # [512, 400]
    fca_b = np.asarray(inputs["fca_b"], f32)
    ATm = fca_w.T.reshape(4, 100, 512).transpose(1, 0, 2).copy()
    A2 = fca_w + fca_b[:, None]
    A2Tm = A2.T.reshape(4, 100, 512).transpose(1, 0, 2).copy()
    baT = fca_b.reshape(4, 128).T.copy()          # [128, 4]

    action = np.asarray(inputs["action"], f32).reshape(1, 400)

    bf = lambda x: np.ascontiguousarray(x).astype(BF16)
    return {
        "w1bd": bf(w1bd), "b1": b1, "w2": bf(w2), "b2": b2,
        "enc": enc, "fceT": bf(fceT), "fceb2": fceb2,
        "fcw2T": bf(fcw2T), "fcw1T": bf(fcw1T), "fcb": fcb,
        "AT": bf(ATm), "A2T": bf(A2Tm), "baT": baT, "F3Tp": bf(F3Tp),
        "act_in": action,
        "ident": np.eye(128, dtype=f32),
        "identb": np.eye(128, dtype=f32).astype(BF16),
    }


_CACHE = {}


def _run(inputs, trace=False):
    from concourse.bass_utils import run_bass_kernel_spmd
    ncores = 8
    if "nc" not in _CACHE:
        _CACHE["nc"] = build_program(ncores=ncores, npairs=8, nsteps=128)
    nc = _CACHE["nc"]
    common = prep_weights(inputs, ncores)
    vf = np.asarray(inputs["video_frame"], np.float32)[0]
    in_maps = []
    for c in range(ncores):
        m = dict(common)
        m["frames"] = np.ascontiguousarray(vf[c::ncores])
        in_maps.append(m)
    return run_bass_kernel_spmd(nc, in_maps, core_ids=list(range(ncores)),
                                trace=trace)


def kernel(**inputs):
    res = _run(inputs, trace=False)
    return res.results[0]["out"].reshape(1, 128, 400).astype(np.float32)



# revision 10
# speedup vs baseline: 1.3553x; 1.0297x over previous
"""Bass/Tile kernel for nn_CustomLSTM: per-frame CNN encode sharded across 8
NeuronCores (round-robin over frames), feats AllGather, then the sequential
softmax-recurrence (the LSTM state is dead code w.r.t. the output) replicated
on every core.

Key structure notes:
- conv1 (3->64, 3x3 s2 p1) runs as a single K=54 matmul per 512-column chunk:
  im2col rows = (frame in pair, ic, ky, kx), built by column-phase compaction
  on DVE (stride-2 cols -> contiguous) + one small SBUF->SBUF DMA per tap
  (partition->free reshape). Two frames share the PE stream via block-diagonal
  weights (M = 2x64 oc).
- conv2 (64->128, 3x3 s2 p1) runs per frame-half as 9 accumulating K=64
  matmuls per 512-column chunk against a zero-padded 130x130 image whose taps
  are plain strided APs. relu+bias+mean-pool fuse into one ACT/DVE op with
  accum_out.
- The recurrence keeps u_t = exp(logits) UNNORMALIZED; A'' = fca_w +
  outer(fca_b, 1) makes A''.u = s*(A.o + b), relu scales, and 1/s is applied
  as the dynamic ACT scale of the next exp. Outputs are normalized at the end
  from R[t] = 1/s_t.
"""
import sys

sys.path.insert(0, "/opt/trn_rl_repo")

import numpy as np
import ml_dtypes

import concourse.bass as bass
import concourse.bacc as bacc
import concourse.mybir as mybir
import concourse.tile as tile

BF16 = ml_dtypes.bfloat16
F32 = mybir.dt.float32
BF = mybir.dt.bfloat16
AF = mybir.ActivationFunctionType
ALU = mybir.AluOpType


def build_program(ncores=8, npairs=8, nsteps=128):
    nfr = 2 * npairs                 # frames per core
    T = ncores * nfr                 # total frames/steps available
    assert nsteps <= T
    nc = bacc.Bacc("TRN2", target_bir_lowering=False, debug=False,
                   num_devices=ncores)

    din = lambda n, s, d: nc.dram_tensor(n, s, d, kind="ExternalInput")
    frames = din("frames", [nfr, 3, 256, 256], F32)
    w1bd = din("w1bd", [54, 128], BF)
    b1 = din("b1", [128, 1], F32)
    w2 = din("w2", [128, 9, 128], BF)
    b2 = din("b2", [128, 1], F32)
    enc = din("enc", [14, 2048], F32)
    fceT = din("fceT", [128, 2, 16, 128], BF)
    fceb2 = din("fceb2", [128, 1], F32)
    fcw2T = din("fcw2T", [128, 7, 400], BF)
    fcw1T = din("fcw1T", [128, 400], BF)
    fcb = din("fcb", [100, 4], F32)
    AT = din("AT", [100, 4, 512], BF)
    A2T = din("A2T", [100, 4, 512], BF)
    baT = din("baT", [128, 4], F32)
    F3Tp = din("F3Tp", [128, 4, 4, 128], BF)
    act_in = din("act_in", [1, 400], F32)
    ident = din("ident", [128, 128], F32)
    identb = din("identb", [128, 128], BF)

    out = nc.dram_tensor("out", [nsteps, 400], F32, kind="ExternalOutput")
    feats_loc = nc.dram_tensor("feats_loc", [128, nfr], F32)
    feats_glob = nc.dram_tensor("feats_glob", [128 * ncores, nfr], F32,
                                addr_space="Shared")

    with tile.TileContext(nc) as tc:
        with tc.tile_pool(name="const", bufs=1) as cp:
            w1bd_t = cp.tile([54, 128], BF)
            nc.sync.dma_start(out=w1bd_t[:], in_=w1bd[:])
            b1_t = cp.tile([128, 1], F32)
            nc.sync.dma_start(out=b1_t[:], in_=b1[:])
            w2_t = cp.tile([128, 9, 128], BF)
            nc.sync.dma_start(out=w2_t[:], in_=w2[:])
            b2_t = cp.tile([128, 1], F32)
            nc.sync.dma_start(out=b2_t[:], in_=b2[:])
            ident_t = cp.tile([128, 128], F32)
            nc.sync.dma_start(out=ident_t[:], in_=ident[:])
            identb_t = cp.tile([128, 128], BF)
            nc.sync.dma_start(out=identb_t[:], in_=identb[:])

            # ---------------- conv stage ----------------
            with tc.tile_pool(name="cv", bufs=2) as cv, \
                 tc.tile_pool(name="cv1", bufs=1) as cv1, \
                 tc.tile_pool(name="ps1", bufs=2, space="PSUM") as ps1, \
                 tc.tile_pool(name="ps2", bufs=4, space="PSUM") as ps2:
                feats_sb = cv1.tile([128, nfr], F32)
                for j in range(npairs):
                    FR = cv.tile([128, 2, 3, 2, 128, 2], F32, tag="FR")
                    for f in range(2):
                        nc.sync.dma_start(out=FR[:, f], in_=frames[2 * j + f].rearrange("c (p r) x -> p c (r x)", r=2))
                    # column-phase compaction (cast f32 -> bf16)
                    Q = {}
                    for f in range(2):
                        for rp in range(2):  # row parity
                            for cpar in range(2):  # col parity
                                q = cv.tile([128, 3, 130], BF,
                                            tag=f"Q{f}{rp}{cpar}")
                                Q[(f, rp, cpar)] = q
                                j0 = cpar  # odd cols shift right by one
                                nc.vector.tensor_copy(
                                    q[:, :, j0:j0 + 128],
                                    FR[:, f, :, rp, :, cpar])
                                if cpar == 1:
                                    nc.vector.memset(q[:, :, 0:1], 0.0)
                    IC = cv.tile([54, 128, 128], BF, tag="IC")
                    nc.vector.memset(IC[:, 0:1, :], 0.0)
                    for f in range(2):
                        for ic in range(3):
                            for ky in range(3):
                                for kx in range(3):
                                    t = 27 * f + 9 * ic + 3 * ky + kx
                                    rp = (ky + 1) % 2  # ky=1 -> even rows
                                    cpar = (kx + 1) % 2
                                    q = Q[(f, rp, cpar)]
                                    j0 = 0 if kx < 2 else 1
                                    if ky == 0:
                                        src = q[0:127, ic, j0:j0 + 128]
                                        dst = IC[t:t + 1, 1:128, :]
                                    else:
                                        src = q[0:128, ic, j0:j0 + 128]
                                        dst = IC[t:t + 1, 0:128, :]
                                    nc.gpsimd.dma_start(out=dst, in_=src)
                    ICf = IC.rearrange("t a b -> t (a b)")
                    c2p = cv1.tile([128, 130, 130], BF, tag="c2p")
                    nc.vector.memset(c2p[:, 0, :], 0.0)
                    nc.vector.memset(c2p[:, 129, :], 0.0)
                    nc.vector.memset(c2p[:, :, 0:1], 0.0)
                    nc.vector.memset(c2p[:, :, 129:130], 0.0)
                    for n in range(32):
                        pm = ps1.tile([128, 512], F32, tag="pm")
                        nc.tensor.matmul(pm[:], w1bd_t[:],
                                         ICf[:, 512 * n:512 * n + 512],
                                         start=True, stop=True)
                        dst = c2p[:, 1 + 4 * n:5 + 4 * n, 1:129]
                        src = pm.rearrange("p (a b) -> p a b", b=128)
                        if n % 2 == 0:
                            nc.scalar.activation(dst, src, AF.Relu,
                                                 bias=b1_t[:, 0:1])
                        else:
                            nc.vector.tensor_scalar(
                                out=dst, in0=src, scalar1=b1_t[:, 0:1],
                                scalar2=0.0, op0=ALU.add, op1=ALU.max)
                    # conv2 per frame half
                    facc = cv.tile([128, 2, 8], F32, tag="facc")
                    for f in range(2):
                        for n2 in range(8):
                            pc = ps2.tile([128, 512], F32, tag="pc")
                            for i, (ky, kx) in enumerate(
                                    [(a, b) for a in range(3)
                                     for b in range(3)]):
                                rhs = c2p[64 * f:64 * f + 64,
                                          ky + 16 * n2:ky + 16 * n2 + 16:2,
                                          kx:kx + 128:2]
                                nc.tensor.matmul(pc[:], w2_t[64*f:64*f+64, 3 * ky + kx, :],
                                                 rhs, start=(i == 0),
                                                 stop=(i == 8))
                            junk = cv.tile([128, 512], BF, tag="junk")
                            if n2 % 2 == 0:
                                nc.scalar.activation(
                                    junk[:], pc[:], AF.Relu,
                                    bias=b2_t[:, 0:1],
                                    accum_out=facc[:, f, n2:n2 + 1])
                            else:
                                nc.vector.tensor_scalar(
                                    out=junk[:], in0=pc[:],
                                    scalar1=b2_t[:, 0:1], scalar2=0.0,
                                    op0=ALU.add, op1=ALU.max,
                                    accum_out=facc[:, f, n2:n2 + 1])
                    for f in range(2):
                        nc.vector.reduce_sum(
                            out=feats_sb[:, 2 * j + f:2 * j + f + 1],
                            in_=facc[:, f, :], axis=mybir.AxisListType.X)
                nc.vector.tensor_scalar_mul(feats_sb[:], feats_sb[:],
                                            1.0 / 4096.0)
                nc.sync.dma_start(out=feats_loc[:], in_=feats_sb[:])
                nc.gpsimd.collective_compute(
                    "AllGather", ALU.bypass,
                    replica_groups=[list(range(ncores))],
                    ins=[feats_loc[:]], outs=[feats_glob[:]])

            # ---------------- ev branch + Lpre ----------------
            sc1 = tc.tile_pool(name="sc1", bufs=1)
            scp = sc1.__enter__()
            Lpre = scp.tile([100, 4, nsteps], F32)
            Lh = scp.tile([100, 4, nsteps], BF)
            Ll = scp.tile([100, 4, nsteps], BF)
            U = scp.tile([100, 4, nsteps], BF)
            ones100 = scp.tile([100, 128], BF)
            nc.vector.memset(ones100[:], 1.0)

            with tc.tile_pool(name="ev", bufs=2) as evp, \
                 tc.tile_pool(name="pse", bufs=2, space="PSUM") as pse, \
                 tc.tile_pool(name="pse1", bufs=1, space="PSUM") as pse1:
                # gather feats -> [128, T] bf16, t = ncores*jloc + c
                fa = evp.tile([128, ncores, nfr], F32)
                nc.sync.dma_start(out=fa[:], in_=feats_glob[:].rearrange(
                    "(c p) j -> p c j", p=128))
                fb = scp.tile([128, T], BF)
                fbv = fb.rearrange("p (j c) -> p c j", c=ncores)
                nc.vector.tensor_copy(fbv, fa[:])

                # encoded_video branch
                enc_sb = evp.tile([14, 2048], F32)
                nc.sync.dma_start(out=enc_sb[:], in_=enc[:])
                fceT_t = evp.tile([128, 2, 16, 128], BF, tag="fceT")
                nc.sync.dma_start(out=fceT_t[:], in_=fceT[:])
                encT = evp.tile([128, 16, 14], BF)
                for k in range(16):
                    pt = pse.tile([128, 14], F32, tag="pt")
                    nc.tensor.transpose(pt[:], enc_sb[:, 128 * k:128 * k + 128],
                                        ident_t[0:14, 0:14])
                    nc.vector.tensor_copy(encT[:, k, :], pt[:])
                pev = pse1.tile([128, 7], F32)
                for k in range(16):
                    for par in range(2):
                        nc.tensor.matmul(
                            pev[:], fceT_t[:, par, k, :],
                            encT[:, k, par:14:2],
                            start=(k == 0 and par == 0),
                            stop=(k == 15 and par == 1))
                fceb2_t = evp.tile([128, 1], F32)
                nc.sync.dma_start(out=fceb2_t[:], in_=fceb2[:])
                ev_sb = evp.tile([128, 7], BF)
                nc.scalar.activation(ev_sb[:], pev[:], AF.Relu,
                                     bias=fceb2_t[:, 0:1])
                fcw2T_t = evp.tile([128, 7, 400], BF, tag="fcw2T")
                nc.sync.dma_start(out=fcw2T_t[:], in_=fcw2T[:])
                pevl = pse1.tile([100, 4], F32)
                for m in range(4):
                    for k in range(7):
                        nc.tensor.matmul(
                            pevl[:, m:m + 1],
                            fcw2T_t[:, k, 100 * m:100 * m + 100],
                            ev_sb[:, k:k + 1], start=(k == 0), stop=(k == 6))
                fcb_t = evp.tile([100, 4], F32)
                nc.sync.dma_start(out=fcb_t[:], in_=fcb[:])
                bconst = evp.tile([100, 4], F32)
                nc.vector.tensor_add(bconst[:], pevl[:], fcb_t[:])
                fcw1T_t = evp.tile([128, 400], BF)
                nc.sync.dma_start(out=fcw1T_t[:], in_=fcw1T[:])
                for m in range(4):
                    plp = pse.tile([100, nsteps], F32, tag="plp")
                    nc.tensor.matmul(plp[:],
                                     fcw1T_t[:, 100 * m:100 * m + 100],
                                     fb[:, 0:nsteps], start=True, stop=True)
                    nc.vector.tensor_scalar_add(Lpre[:, m, :], plp[:],
                                                bconst[:, m:m + 1])
                    nc.vector.tensor_copy(Lh[0:100, m, :], Lpre[0:100, m, :])
                    nc.vector.tensor_sub(Ll[0:100, m, :], Lpre[0:100, m, :],
                                         Lh[0:100, m, :])

            # ---------------- scan ----------------
            AT_t = scp.tile([100, 4, 512], BF)
            nc.sync.dma_start(out=AT_t[:], in_=AT[:])
            A2T_t = scp.tile([100, 4, 512], BF)
            nc.sync.dma_start(out=A2T_t[:], in_=A2T[:])
            baT_t = scp.tile([128, 4], F32)
            nc.sync.dma_start(out=baT_t[:], in_=baT[:])
            F3Tp_t = scp.tile([128, 4, 4, 128], BF)
            nc.sync.dma_start(out=F3Tp_t[:], in_=F3Tp[:])

            with tc.tile_pool(name="sstep", bufs=3) as ssp, \
                 tc.tile_pool(name="psA", bufs=2, space="PSUM") as psA, \
                 tc.tile_pool(name="psL", bufs=2, space="PSUM") as psL, \
                 tc.tile_pool(name="psS", bufs=2, space="PSUM") as psS:
                # step 0 input: transpose action row into [100, 4]
                act_row = ssp.tile([1, 400], F32, tag="actrow")
                nc.sync.dma_start(out=act_row[:], in_=act_in[:])
                u0 = ssp.tile([100, 4], BF, tag="u0")
                for m in range(4):
                    pa0 = psS.tile([128, 4], F32, tag="S")
                    nc.tensor.transpose(pa0[0:100, 0:1],
                                        act_row[0:1, 100 * m:100 * m + 100],
                                        ident_t[0:1, 0:1])
                    nc.vector.tensor_copy(u0[:, m:m + 1], pa0[0:100, 0:1])

                for t in range(nsteps):
                    Wmat = AT_t if t == 0 else A2T_t
                    ucur = (lambda k: u0[:, k:k + 1]) if t == 0 else \
                        (lambda k, tt=t - 1: U[0:100, k, tt:tt + 1])
                    # normalizer: S = colsum(u_{t-1}) broadcast to 128 rows
                    if t > 0:
                        S_ps = psS.tile([128, 4], F32, tag="S")
                        nc.tensor.matmul(S_ps[:], ones100[:],
                                         U[0:100, :, t - 1],
                                         start=True, stop=True)
                    a_ps = psA.tile([128, 4], F32, tag="a_ps")
                    for m in range(4):
                        for k in range(4):
                            nc.tensor.matmul(
                                a_ps[:, m:m + 1],
                                Wmat[:, k, 128 * m:128 * m + 128],
                                ucur(k), start=(k == 0), stop=(k == 3))
                    w_sb = ssp.tile([128, 4], BF, tag="w_sb")
                    if t == 0:
                        for m in range(4):
                            nc.vector.tensor_scalar(
                                out=w_sb[:, m:m + 1], in0=a_ps[:, m:m + 1],
                                scalar1=baT_t[:, m:m + 1], scalar2=0.0,
                                op0=ALU.add, op1=ALU.max)
                    else:
                        s1 = ssp.tile([128, 1], F32, tag="s1")
                        nc.vector.reduce_sum(out=s1[:], in_=S_ps[:],
                                             axis=mybir.AxisListType.X)
                        r_t = ssp.tile([128, 1], F32, tag="r_t")
                        nc.vector.reciprocal(r_t[:], s1[:])
                        nc.vector.tensor_scalar(
                            out=w_sb[:], in0=a_ps[:],
                            scalar1=r_t[:, 0:1], scalar2=0.0,
                            op0=ALU.mult, op1=ALU.max)
                    # logits = F3.w + Lpre (hi+lo), all in one PSUM group
                    l_ps = psL.tile([128, 4], F32, tag="l_ps")
                    nc.tensor.matmul(l_ps[:], identb_t[0:100, :],
                                     Lh[0:100, :, t], start=True, stop=False)
                    nc.tensor.matmul(l_ps[:], identb_t[0:100, :],
                                     Ll[0:100, :, t], start=False, stop=False)
                    for m in range(4):
                        for k in range(4):
                            nc.tensor.matmul(
                                l_ps[:, m:m + 1],
                                F3Tp_t[:, k, m, :],
                                w_sb[:, k:k + 1], start=False,
                                stop=(k == 3))
                    nc.scalar.activation(U[0:100, :, t], l_ps[0:100, :],
                                         AF.Exp)

                # final: transpose U to [t, c] layout and normalize rows
                OT = ssp.tile([128, 400], F32, tag="OT")
                for m in range(4):
                    tp_ps = psA.tile([128, 100], BF, tag="tp_ps")
                    nc.tensor.transpose(tp_ps[0:nsteps, :],
                                        U[0:100, m, 0:nsteps],
                                        identb_t[0:100, 0:100])
                    nc.vector.tensor_copy(
                        OT[0:nsteps, 100 * m:100 * m + 100],
                        tp_ps[0:nsteps, :])
                rowsum = ssp.tile([128, 1], F32, tag="rowsum")
                nc.vector.reduce_sum(out=rowsum[0:nsteps],
                                     in_=OT[0:nsteps, :],
                                     axis=mybir.AxisListType.X)
                rinv = ssp.tile([128, 1], F32, tag="rinv")
                nc.vector.reciprocal(rinv[0:nsteps], rowsum[0:nsteps])
                OTn = ssp.tile([128, 400], F32, tag="OTn")
                nc.vector.tensor_scalar_mul(OTn[0:nsteps, :],
                                            OT[0:nsteps, :],
                                            rinv[0:nsteps, 0:1])
                nc.sync.dma_start(out=out[:], in_=OTn[0:nsteps, :])
            sc1.__exit__(None, None, None)

    nc.compile()
    return nc


def prep_weights(inputs, ncores=8):
    """Host-side numpy prep of all weight layouts. Returns dict of arrays
    shared by all cores (frames excluded)."""
    f32 = np.float32
    conv1_w = np.asarray(inputs["conv1_w"], f32)
    conv2_w = np.asarray(inputs["conv2_w"], f32)
    w1bd = np.zeros((54, 128), f32)
    for f in range(2):
        for ic in range(3):
            for ky in range(3):
                for kx in range(3):
                    t = 27 * f + 9 * ic + 3 * ky + kx
                    w1bd[t, 64 * f:64 * f + 64] = conv1_w[:, ic, ky, kx]
    b1 = np.tile(np.asarray(inputs["conv1_b"], f32), 2).reshape(128, 1)
    w2 = np.transpose(conv2_w, (1, 2, 3, 0)).reshape(64, 9, 128)
    w2 = np.concatenate([w2, w2], axis=0)
    b2 = np.asarray(inputs["conv2_b"], f32).reshape(128, 1)

    enc = np.asarray(inputs["encoded_video"], f32).reshape(14, 2048)
    fce_w = np.asarray(inputs["fce_w"], f32)      # [64, 2048]
    fceT = np.zeros((128, 2, 16, 128), f32)
    fceTf = fce_w.T.reshape(16, 128, 64)      # [k, p, o]
    fceT[:, 0, :, 0:64] = fceTf.transpose(1, 0, 2)
    fceT[:, 1, :, 64:128] = fceTf.transpose(1, 0, 2)
    fceb2 = np.tile(np.asarray(inputs["fce_b"], f32), 2).reshape(128, 1)

    fc_w = np.asarray(inputs["fc_w"], f32)        # [400, 1536]
    fc_b = np.asarray(inputs["fc_b"], f32)
    fcw1T = fc_w[:, 0:128].T.copy()               # [128, 400]
    fcw2T = fc_w[:, 128:1024].T.reshape(7, 128, 400).transpose(1, 0, 2).copy()
    F3t = fc_w[:, 1024:1536].T                    # [512, 400]
    F3Tp = np.zeros((128, 4, 4, 128), f32)
    for k in range(4):
        for m in range(4):
            F3Tp[:, k, m, 0:100] = F3t[128 * k:128 * k + 128,
                                       100 * m:100 * m + 100]
    fcb = fc_b.reshape(4, 100).T.copy()           # [100, 4]

    fca_w = np.asarray(inputs["fca_w"], f32)      # [512, 400]
    fca_b = np.asarray(inputs["fca_b"], f32)
    ATm = fca_w.T.reshape(4, 100, 512).transpose(1, 0, 2).copy()
    A2 = fca_w + fca_b[:, None]
    A2Tm = A2.T.reshape(4, 100, 512).transpose(1, 0, 2).copy()
    baT = fca_b.reshape(4, 128).T.copy()          # [128, 4]

    action = np.asarray(inputs["action"], f32).reshape(1, 400)

    bf = lambda x: np.ascontiguousarray(x).astype(BF16)
    return {
        "w1bd": bf(w1bd), "b1": b1, "w2": bf(w2), "b2": b2,
        "enc": enc, "fceT": bf(fceT), "fceb2": fceb2,
        "fcw2T": bf(fcw2T), "fcw1T": bf(fcw1T), "fcb": fcb,
        "AT": bf(ATm), "A2T": bf(A2Tm), "baT": baT, "F3Tp": bf(F3Tp),
        "act_in": action,
        "ident": np.eye(128, dtype=f32),
        "identb": np.eye(128, dtype=f32).astype(BF16),
    }


_CACHE = {}


def _run(inputs, trace=False):
    from concourse.bass_utils import run_bass_kernel_spmd
    ncores = 8
    if "nc" not in _CACHE:
        _CACHE["nc"] = build_program(ncores=ncores, npairs=8, nsteps=128)
    nc = _CACHE["nc"]
    common = prep_weights(inputs, ncores)
    vf = np.asarray(inputs["video_frame"], np.float32)[0]
    in_maps = []
    for c in range(ncores):
        m = dict(common)
        m["frames"] = np.ascontiguousarray(vf[c::ncores])
        in_maps.append(m)
    return run_bass_kernel_spmd(nc, in_maps, core_ids=list(range(ncores)),
                                trace=trace)


def kernel(**inputs):
    res = _run(inputs, trace=False)
    return res.results[0]["out"].reshape(1, 128, 400).astype(np.float32)



# revision 24
# speedup vs baseline: 1.8187x; 1.3419x over previous
"""Bass/Tile kernel for nn_CustomLSTM: per-frame CNN encode sharded across 8
NeuronCores (round-robin over frames), feats AllGather, then the sequential
softmax-recurrence (the LSTM state is dead code w.r.t. the output) replicated
on every core.

Key structure notes:
- conv1 (3->64, 3x3 s2 p1) runs as a single K=54 matmul per 512-column chunk:
  im2col rows = (frame in pair, ic, ky, kx), built by column-phase compaction
  on DVE (stride-2 cols -> contiguous) + one small SBUF->SBUF DMA per tap
  (partition->free reshape). Two frames share the PE stream via block-diagonal
  weights (M = 2x64 oc).
- conv2 (64->128, 3x3 s2 p1) runs per frame-half as 9 accumulating K=64
  matmuls per 512-column chunk against a zero-padded 130x130 image whose taps
  are plain strided APs. relu+bias+mean-pool fuse into one ACT/DVE op with
  accum_out.
- The recurrence keeps u_t = exp(logits) UNNORMALIZED; A'' = fca_w +
  outer(fca_b, 1) makes A''.u = s*(A.o + b), relu scales, and 1/s is applied
  as the dynamic ACT scale of the next exp. Outputs are normalized at the end
  from R[t] = 1/s_t.
"""
import sys

sys.path.insert(0, "/opt/trn_rl_repo")

import numpy as np
import ml_dtypes

import concourse.bass as bass
import concourse.bacc as bacc
import concourse.mybir as mybir
import concourse.tile as tile

BF16 = ml_dtypes.bfloat16
F8E4 = ml_dtypes.float8_e4m3
F32 = mybir.dt.float32
BF = mybir.dt.bfloat16
FP8 = mybir.dt.float8e4
AF = mybir.ActivationFunctionType
ALU = mybir.AluOpType
DRM = mybir.MatmulPerfMode.DoubleRow
# conv scaling: c2p holds 8x the true conv1 output (fp8 range), w2 is
# stored 32x (fp8 range); the conv2 output activation folds 1/(8*32) and
# the 1/4096 mean-pool divisor into its scale.
SA = 1.0 / (4096.0 * 8.0 * 32.0)


def build_program(ncores=8, npairs=8, nsteps=128):
    nfr = 2 * npairs                 # frames per core
    T = ncores * nfr                 # total frames/steps available
    assert nsteps <= T
    nc = bacc.Bacc("TRN2", target_bir_lowering=False, debug=False,
                   num_devices=ncores)

    din = lambda n, s, d: nc.dram_tensor(n, s, d, kind="ExternalInput")
    frames = din("frames", [nfr, 3, 256, 256], F32)
    w1bd = din("w1bd", [54, 128], FP8)
    b1 = din("b1", [128, 1], F32)
    w2f8 = din("w2f8", [128, 5, 2, 128], FP8)
    b2s = din("b2s", [128, 1], F32)
    enc = din("enc", [14, 2048], F32)
    fceT = din("fceT", [128, 2, 16, 128], BF)
    fceb2 = din("fceb2", [128, 1], F32)
    fcw2T = din("fcw2T", [128, 7, 400], BF)
    fcw1T = din("fcw1T", [128, 400], BF)
    fcb = din("fcb", [100, 4], F32)
    AT = din("AT", [100, 4, 512], BF)
    A2T = din("A2T", [100, 4, 512], BF)
    baT = din("baT", [128, 4], F32)
    F3Tp = din("F3Tp", [128, 4, 4, 128], BF)
    act_in = din("act_in", [1, 400], F32)
    ident = din("ident", [128, 128], F32)
    identb = din("identb", [128, 128], BF)

    out = nc.dram_tensor("out", [nsteps, 400], F32, kind="ExternalOutput")
    feats_loc = nc.dram_tensor("feats_loc", [128, nfr], F32)
    feats_glob = nc.dram_tensor("feats_glob", [128 * ncores, nfr], F32,
                                addr_space="Shared")

    with tile.TileContext(nc) as tc:
        with tc.tile_pool(name="const", bufs=1) as cp:
            w1bd_t = cp.tile([54, 128], FP8)
            nc.sync.dma_start(out=w1bd_t[:], in_=w1bd[:])
            b1_t = cp.tile([128, 1], F32)
            nc.sync.dma_start(out=b1_t[:], in_=b1[:])
            w2f8_t = cp.tile([128, 5, 2, 128], FP8)
            nc.sync.dma_start(out=w2f8_t[:], in_=w2f8[:])
            b2s_t = cp.tile([128, 1], F32)
            nc.sync.dma_start(out=b2s_t[:], in_=b2s[:])
            ident_t = cp.tile([128, 128], F32)
            nc.sync.dma_start(out=ident_t[:], in_=ident[:])
            identb_t = cp.tile([128, 128], BF)
            nc.sync.dma_start(out=identb_t[:], in_=identb[:])

            # ---------------- conv stage ----------------
            with tc.tile_pool(name="cv", bufs=2) as cv, \
                 tc.tile_pool(name="cv1", bufs=1) as cv1, \
                 tc.tile_pool(name="ps1", bufs=3, space="PSUM") as ps1, \
                 tc.tile_pool(name="ps2", bufs=5, space="PSUM") as ps2:
                feats_sb = cv1.tile([128, nfr], F32)
                for j in range(npairs):
                    FR = cv.tile([128, 2, 3, 2, 128, 2], F32, tag="FR")
                    for f in range(2):
                        feng = nc.sync if f == 0 else nc.scalar
                        feng.dma_start(out=FR[:, f], in_=frames[2 * j + f].rearrange("c (p r) x -> p c (r x)", r=2))
                    # column-phase compaction (cast f32 -> fp8e4)
                    Q = {}
                    for f in range(2):
                        for rp in range(2):  # row parity
                            for cpar in range(2):  # col parity
                                q = cv.tile([128, 3, 130], FP8,
                                            tag=f"Q{f}{rp}{cpar}")
                                Q[(f, rp, cpar)] = q
                                j0 = cpar  # odd cols shift right by one
                                nc.vector.tensor_copy(
                                    q[:, :, j0:j0 + 128],
                                    FR[:, f, :, rp, :, cpar])
                                if cpar == 1:
                                    nc.vector.memset(q[:, :, 0:1], 0.0)
                    IC = cv.tile([54, 128, 128], FP8, tag="IC")
                    nc.vector.memset(IC[:, 0:1, :], 0.0)
                    qi = 0
                    for f in range(2):
                        for ic in range(3):
                            for ky in range(3):
                                for kx in range(3):
                                    t = 27 * f + 9 * ic + 3 * ky + kx
                                    rp = (ky + 1) % 2  # ky=1 -> even rows
                                    cpar = (kx + 1) % 2
                                    q = Q[(f, rp, cpar)]
                                    j0 = 0 if kx < 2 else 1
                                    if ky == 0:
                                        src = q[0:127, ic, j0:j0 + 128]
                                        dst = IC[t:t + 1, 1:128, :]
                                    else:
                                        src = q[0:128, ic, j0:j0 + 128]
                                        dst = IC[t:t + 1, 0:128, :]
                                    if j == 0:
                                        eng = (nc.sync, nc.gpsimd,
                                               nc.scalar)[qi % 3]
                                    else:
                                        eng = nc.gpsimd if qi % 2 \
                                            else nc.sync
                                    eng.dma_start(out=dst, in_=src)
                                    qi += 1
                    ICf = IC.rearrange("t a b -> t (a b)")
                    c2p = cv1.tile([128, 130, 130], FP8, tag="c2p")
                    nc.vector.memset(c2p[:, 0, :], 0.0)
                    nc.vector.memset(c2p[:, 129, :], 0.0)
                    nc.vector.memset(c2p[:, :, 0:1], 0.0)
                    nc.vector.memset(c2p[:, :, 129:130], 0.0)
                    for n in range(32):
                        pm = ps1.tile([128, 512], F32, tag="pm")
                        nc.tensor.matmul(pm[:], w1bd_t[:],
                                         ICf[:, 512 * n:512 * n + 512],
                                         start=True, stop=True)
                        dst = c2p[:, 1 + 4 * n:5 + 4 * n, 1:129]
                        src = pm.rearrange("p (a b) -> p a b", b=128)
                        if n % 2 == 0:
                            nc.scalar.activation(dst, src, AF.Relu,
                                                 bias=b1_t[:, 0:1])
                        else:
                            nc.vector.tensor_scalar(
                                out=dst, in0=src, scalar1=b1_t[:, 0:1],
                                scalar2=0.0, op0=ALU.add, op1=ALU.max)
                    # conv2 per frame half: 4 DoubleRow tap-pair matmuls +
                    # one single-tap matmul per 512-column chunk
                    facc = cv.tile([128, 2, 8], F32, tag="facc")
                    for f in range(2):
                        for n2 in range(8):
                            pc = ps2.tile([128, 512], F32, tag="pc")
                            base = 16 * n2
                            c2f = c2p[64 * f:64 * f + 64]
                            for s in range(3):  # taps (s,0)+(s,1)
                                rhs = c2f[:, s + base:s + base + 16:2,
                                          0:128].rearrange(
                                    "p i (c two) -> p two i c", two=2)
                                nc.tensor.matmul(pc[:], w2f8_t[64 * f:64 * f + 64, s],
                                                 rhs, start=(s == 0),
                                                 stop=False, perf_mode=DRM)
                            # taps (0,2)+(1,2) paired on row parity
                            rhs = c2f[:, base:base + 16,
                                      2:130:2].rearrange(
                                "p (i two) c -> p two i c", two=2)
                            nc.tensor.matmul(pc[:], w2f8_t[64 * f:64 * f + 64, 3], rhs,
                                             start=False, stop=False,
                                             perf_mode=DRM)
                            # tap (2,2) single
                            rhs = c2f[:, 2 + base:2 + base + 16:2, 2:130:2]
                            nc.tensor.matmul(pc[:], w2f8_t[64 * f:64 * f + 64, 4, 0], rhs,
                                             start=False, stop=True)
                            junk = cv.tile([128, 512], FP8, tag="junk")
                            if n2 % 2 == 0:
                                nc.scalar.activation(
                                    junk[:], pc[:], AF.Relu,
                                    bias=b2s_t[:, 0:1], scale=SA,
                                    accum_out=facc[:, f, n2:n2 + 1])
                            else:
                                nc.vector.tensor_scalar(
                                    out=junk[:], in0=pc[:],
                                    scalar1=SA, scalar2=0.0,
                                    op0=ALU.mult, op1=ALU.max,
                                    accum_out=facc[:, f, n2:n2 + 1])
                    for f in range(2):
                        nc.vector.reduce_sum(
                            out=feats_sb[:, 2 * j + f:2 * j + f + 1],
                            in_=facc[:, f, :], axis=mybir.AxisListType.X)
                nc.sync.dma_start(out=feats_loc[:], in_=feats_sb[:])
                nc.gpsimd.collective_compute(
                    "AllGather", ALU.bypass,
                    replica_groups=[list(range(ncores))],
                    ins=[feats_loc[:]], outs=[feats_glob[:]])

            # ---------------- ev branch + Lpre ----------------
            sc1 = tc.tile_pool(name="sc1", bufs=1)
            scp = sc1.__enter__()
            Lpre = scp.tile([100, 4, nsteps], F32)
            Lh = scp.tile([100, 4, nsteps], BF)
            Ll = scp.tile([100, 4, nsteps], BF)
            U = scp.tile([100, 4, nsteps], BF)
            ones100 = scp.tile([100, 128], BF)
            nc.vector.memset(ones100[:], 1.0)

            with tc.tile_pool(name="ev", bufs=2) as evp, \
                 tc.tile_pool(name="pse", bufs=2, space="PSUM") as pse, \
                 tc.tile_pool(name="pse1", bufs=1, space="PSUM") as pse1:
                # gather feats -> [128, T] bf16, t = ncores*jloc + c
                fa = evp.tile([128, ncores, nfr], F32)
                nc.sync.dma_start(out=fa[:], in_=feats_glob[:].rearrange(
                    "(c p) j -> p c j", p=128))
                fb = scp.tile([128, T], BF)
                fbv = fb.rearrange("p (j c) -> p c j", c=ncores)
                nc.vector.tensor_copy(fbv, fa[:])

                # encoded_video branch
                enc_sb = evp.tile([14, 2048], F32)
                nc.sync.dma_start(out=enc_sb[:], in_=enc[:])
                fceT_t = evp.tile([128, 2, 16, 128], BF, tag="fceT")
                nc.sync.dma_start(out=fceT_t[:], in_=fceT[:])
                encT = evp.tile([128, 16, 14], BF)
                for k in range(16):
                    pt = pse.tile([128, 14], F32, tag="pt")
                    nc.tensor.transpose(pt[:], enc_sb[:, 128 * k:128 * k + 128],
                                        ident_t[0:14, 0:14])
                    nc.vector.tensor_copy(encT[:, k, :], pt[:])
                pev = pse1.tile([128, 7], F32)
                for k in range(16):
                    for par in range(2):
                        nc.tensor.matmul(
                            pev[:], fceT_t[:, par, k, :],
                            encT[:, k, par:14:2],
                            start=(k == 0 and par == 0),
                            stop=(k == 15 and par == 1))
                fceb2_t = evp.tile([128, 1], F32)
                nc.sync.dma_start(out=fceb2_t[:], in_=fceb2[:])
                ev_sb = evp.tile([128, 7], BF)
                nc.scalar.activation(ev_sb[:], pev[:], AF.Relu,
                                     bias=fceb2_t[:, 0:1])
                fcw2T_t = evp.tile([128, 7, 400], BF, tag="fcw2T")
                nc.sync.dma_start(out=fcw2T_t[:], in_=fcw2T[:])
                pevl = pse1.tile([100, 4], F32)
                for m in range(4):
                    for k in range(7):
                        nc.tensor.matmul(
                            pevl[:, m:m + 1],
                            fcw2T_t[:, k, 100 * m:100 * m + 100],
                            ev_sb[:, k:k + 1], start=(k == 0), stop=(k == 6))
                fcb_t = evp.tile([100, 4], F32)
                nc.sync.dma_start(out=fcb_t[:], in_=fcb[:])
                bconst = evp.tile([100, 4], F32)
                nc.vector.tensor_add(bconst[:], pevl[:], fcb_t[:])
                fcw1T_t = evp.tile([128, 400], BF)
                nc.sync.dma_start(out=fcw1T_t[:], in_=fcw1T[:])
                for m in range(4):
                    plp = pse.tile([100, nsteps], F32, tag="plp")
                    nc.tensor.matmul(plp[:],
                                     fcw1T_t[:, 100 * m:100 * m + 100],
                                     fb[:, 0:nsteps], start=True, stop=True)
                    nc.vector.tensor_scalar_add(Lpre[:, m, :], plp[:],
                                                bconst[:, m:m + 1])
                    nc.vector.tensor_copy(Lh[0:100, m, :], Lpre[0:100, m, :])
                    nc.vector.tensor_sub(Ll[0:100, m, :], Lpre[0:100, m, :],
                                         Lh[0:100, m, :])

            # ---------------- scan ----------------
            AT_t = scp.tile([100, 4, 512], BF)
            nc.sync.dma_start(out=AT_t[:], in_=AT[:])
            A2T_t = scp.tile([100, 4, 512], BF)
            nc.sync.dma_start(out=A2T_t[:], in_=A2T[:])
            baT_t = scp.tile([128, 4], F32)
            nc.sync.dma_start(out=baT_t[:], in_=baT[:])
            F3Tp_t = scp.tile([128, 4, 4, 128], BF)
            nc.sync.dma_start(out=F3Tp_t[:], in_=F3Tp[:])

            with tc.tile_pool(name="sstep", bufs=3) as ssp, \
                 tc.tile_pool(name="psA", bufs=2, space="PSUM") as psA, \
                 tc.tile_pool(name="psL", bufs=2, space="PSUM") as psL, \
                 tc.tile_pool(name="psS", bufs=2, space="PSUM") as psS:
                # step 0 input: transpose action row into [100, 4]
                act_row = ssp.tile([1, 400], F32, tag="actrow")
                nc.sync.dma_start(out=act_row[:], in_=act_in[:])
                u0 = ssp.tile([100, 4], BF, tag="u0")
                for m in range(4):
                    pa0 = psS.tile([128, 4], F32, tag="S")
                    nc.tensor.transpose(pa0[0:100, 0:1],
                                        act_row[0:1, 100 * m:100 * m + 100],
                                        ident_t[0:1, 0:1])
                    nc.vector.tensor_copy(u0[:, m:m + 1], pa0[0:100, 0:1])

                for t in range(nsteps):
                    Wmat = AT_t if t == 0 else A2T_t
                    ucur = (lambda k: u0[:, k:k + 1]) if t == 0 else \
                        (lambda k, tt=t - 1: U[0:100, k, tt:tt + 1])
                    # normalizer: S = colsum(u_{t-1}) broadcast to 128 rows
                    if t > 0:
                        S_ps = psS.tile([128, 4], F32, tag="S")
                        nc.tensor.matmul(S_ps[:], ones100[:],
                                         U[0:100, :, t - 1],
                                         start=True, stop=True)
                    a_ps = psA.tile([128, 4], F32, tag="a_ps")
                    for m in range(4):
                        for k in range(4):
                            nc.tensor.matmul(
                                a_ps[:, m:m + 1],
                                Wmat[:, k, 128 * m:128 * m + 128],
                                ucur(k), start=(k == 0), stop=(k == 3))
                    w_sb = ssp.tile([128, 4], BF, tag="w_sb")
                    if t == 0:
                        for m in range(4):
                            nc.vector.tensor_scalar(
                                out=w_sb[:, m:m + 1], in0=a_ps[:, m:m + 1],
                                scalar1=baT_t[:, m:m + 1], scalar2=0.0,
                                op0=ALU.add, op1=ALU.max)
                    else:
                        s1 = ssp.tile([128, 1], F32, tag="s1")
                        nc.vector.reduce_sum(out=s1[:], in_=S_ps[:],
                                             axis=mybir.AxisListType.X)
                        r_t = ssp.tile([128, 1], F32, tag="r_t")
                        nc.vector.reciprocal(r_t[:], s1[:])
                        nc.vector.tensor_scalar(
                            out=w_sb[:], in0=a_ps[:],
                            scalar1=r_t[:, 0:1], scalar2=0.0,
                            op0=ALU.mult, op1=ALU.max)
                    # logits = F3.w + Lpre (hi+lo), all in one PSUM group
                    l_ps = psL.tile([128, 4], F32, tag="l_ps")
                    nc.tensor.matmul(l_ps[:], identb_t[0:100, :],
                                     Lh[0:100, :, t], start=True, stop=False)
                    nc.tensor.matmul(l_ps[:], identb_t[0:100, :],
                                     Ll[0:100, :, t], start=False, stop=False)
                    for m in range(4):
                        for k in range(4):
                            nc.tensor.matmul(
                                l_ps[:, m:m + 1],
                                F3Tp_t[:, k, m, :],
                                w_sb[:, k:k + 1], start=False,
                                stop=(k == 3))
                    nc.scalar.activation(U[0:100, :, t], l_ps[0:100, :],
                                         AF.Exp)

                # final: transpose U to [t, c] layout and normalize rows
                OT = ssp.tile([128, 400], F32, tag="OT")
                for m in range(4):
                    tp_ps = psA.tile([128, 100], BF, tag="tp_ps")
                    nc.tensor.transpose(tp_ps[0:nsteps, :],
                                        U[0:100, m, 0:nsteps],
                                        identb_t[0:100, 0:100])
                    nc.vector.tensor_copy(
                        OT[0:nsteps, 100 * m:100 * m + 100],
                        tp_ps[0:nsteps, :])
                rowsum = ssp.tile([128, 1], F32, tag="rowsum")
                nc.vector.reduce_sum(out=rowsum[0:nsteps],
                                     in_=OT[0:nsteps, :],
                                     axis=mybir.AxisListType.X)
                rinv = ssp.tile([128, 1], F32, tag="rinv")
                nc.vector.reciprocal(rinv[0:nsteps], rowsum[0:nsteps])
                OTn = ssp.tile([128, 400], F32, tag="OTn")
                nc.vector.tensor_scalar_mul(OTn[0:nsteps, :],
                                            OT[0:nsteps, :],
                                            rinv[0:nsteps, 0:1])
                nc.sync.dma_start(out=out[:], in_=OTn[0:nsteps, :])
            sc1.__exit__(None, None, None)

    nc.compile()
    return nc


def prep_weights(inputs, ncores=8):
    """Host-side numpy prep of all weight layouts. Returns dict of arrays
    shared by all cores (frames excluded)."""
    f32 = np.float32
    conv1_w = np.asarray(inputs["conv1_w"], f32)
    conv2_w = np.asarray(inputs["conv2_w"], f32)
    w1bd = np.zeros((54, 128), f32)
    for f in range(2):
        for ic in range(3):
            for ky in range(3):
                for kx in range(3):
                    t = 27 * f + 9 * ic + 3 * ky + kx
                    w1bd[t, 64 * f:64 * f + 64] = conv1_w[:, ic, ky, kx]
    w1bd *= 8.0  # c2p holds 8x conv1 output for fp8 range
    b1 = np.tile(np.asarray(inputs["conv1_b"], f32) * 8.0, 2).reshape(128, 1)
    # conv2 weights as DoubleRow tap-pair tiles, 32x scaled for fp8 range
    w2t = conv2_w.transpose(1, 2, 3, 0)           # [ic, ky, kx, oc]
    w2f8 = np.zeros((64, 5, 2, 128), f32)
    for s in range(3):
        w2f8[:, s, 0, :] = w2t[:, s, 0, :]
        w2f8[:, s, 1, :] = w2t[:, s, 1, :]
    w2f8[:, 3, 0, :] = w2t[:, 0, 2, :]
    w2f8[:, 3, 1, :] = w2t[:, 1, 2, :]
    w2f8[:, 4, 0, :] = w2t[:, 2, 2, :]
    w2f8 *= 32.0
    w2f8 = np.concatenate([w2f8, w2f8], axis=0)  # both partition halves
    b2s = (np.asarray(inputs["conv2_b"], f32) / 4096.0).reshape(128, 1)

    enc = np.asarray(inputs["encoded_video"], f32).reshape(14, 2048)
    fce_w = np.asarray(inputs["fce_w"], f32)      # [64, 2048]
    fceT = np.zeros((128, 2, 16, 128), f32)
    fceTf = fce_w.T.reshape(16, 128, 64)      # [k, p, o]
    fceT[:, 0, :, 0:64] = fceTf.transpose(1, 0, 2)
    fceT[:, 1, :, 64:128] = fceTf.transpose(1, 0, 2)
    fceb2 = np.tile(np.asarray(inputs["fce_b"], f32), 2).reshape(128, 1)

    fc_w = np.asarray(inputs["fc_w"], f32)        # [400, 1536]
    fc_b = np.asarray(inputs["fc_b"], f32)
    fcw1T = fc_w[:, 0:128].T.copy()               # [128, 400]
    fcw2T = fc_w[:, 128:1024].T.reshape(7, 128, 400).transpose(1, 0, 2).copy()
    F3t = fc_w[:, 1024:1536].T                    # [512, 400]
    F3Tp = np.zeros((128, 4, 4, 128), f32)
    for k in range(4):
        for m in range(4):
            F3Tp[:, k, m, 0:100] = F3t[128 * k:128 * k + 128,
                                       100 * m:100 * m + 100]
    fcb = fc_b.reshape(4, 100).T.copy()           # [100, 4]

    fca_w = np.asarray(inputs["fca_w"], f32)      # [512, 400]
    fca_b = np.asarray(inputs["fca_b"], f32)
    ATm = fca_w.T.reshape(4, 100, 512).transpose(1, 0, 2).copy()
    A2 = fca_w + fca_b[:, None]
    A2Tm = A2.T.reshape(4, 100, 512).transpose(1, 0, 2).copy()
    baT = fca_b.reshape(4, 128).T.copy()          # [128, 4]

    action = np.asarray(inputs["action"], f32).reshape(1, 400)

    bf = lambda x: np.ascontiguousarray(x).astype(BF16)
    f8 = lambda x: np.ascontiguousarray(x).astype(F8E4)
    return {
        "w1bd": f8(w1bd), "b1": b1, "w2f8": f8(w2f8), "b2s": b2s,
        "enc": enc, "fceT": bf(fceT), "fceb2": fceb2,
        "fcw2T": bf(fcw2T), "fcw1T": bf(fcw1T), "fcb": fcb,
        "AT": bf(ATm), "A2T": bf(A2Tm), "baT": baT, "F3Tp": bf(F3Tp),
        "act_in": action,
        "ident": np.eye(128, dtype=f32),
        "identb": np.eye(128, dtype=f32).astype(BF16),
    }


_CACHE = {}


def _run(inputs, trace=False):
    from concourse.bass_utils import run_bass_kernel_spmd
    ncores = 8
    if "nc" not in _CACHE:
        _CACHE["nc"] = build_program(ncores=ncores, npairs=8, nsteps=128)
    nc = _CACHE["nc"]
    common = prep_weights(inputs, ncores)
    vf = np.asarray(inputs["video_frame"], np.float32)[0]
    in_maps = []
    for c in range(ncores):
        m = dict(common)
        m["frames"] = np.ascontiguousarray(vf[c::ncores])
        in_maps.append(m)
    return run_bass_kernel_spmd(nc, in_maps, core_ids=list(range(ncores)),
                                trace=trace)


def kernel(**inputs):
    res = _run(inputs, trace=False)
    return res.results[0]["out"].reshape(1, 128, 400).astype(np.float32)



# revision 27
# speedup vs baseline: 1.8819x; 1.0348x over previous
"""Bass/Tile kernel for nn_CustomLSTM: per-frame CNN encode sharded across 8
NeuronCores (round-robin over frames), feats AllGather, then the sequential
softmax-recurrence (the LSTM state is dead code w.r.t. the output) replicated
on every core.

Key structure notes:
- conv1 (3->64, 3x3 s2 p1) runs as a single K=54 matmul per 512-column chunk:
  im2col rows = (frame in pair, ic, ky, kx), built by column-phase compaction
  on DVE (stride-2 cols -> contiguous) + one small SBUF->SBUF DMA per tap
  (partition->free reshape). Two frames share the PE stream via block-diagonal
  weights (M = 2x64 oc).
- conv2 (64->128, 3x3 s2 p1) runs per frame-half as 9 accumulating K=64
  matmuls per 512-column chunk against a zero-padded 130x130 image whose taps
  are plain strided APs. relu+bias+mean-pool fuse into one ACT/DVE op with
  accum_out.
- The recurrence keeps u_t = exp(logits) UNNORMALIZED; A'' = fca_w +
  outer(fca_b, 1) makes A''.u = s*(A.o + b), relu scales, and 1/s is applied
  as the dynamic ACT scale of the next exp. Outputs are normalized at the end
  from R[t] = 1/s_t.
"""
import sys

sys.path.insert(0, "/opt/trn_rl_repo")

import numpy as np
import ml_dtypes

import concourse.bass as bass
import concourse.bacc as bacc
import concourse.mybir as mybir
import concourse.tile as tile

BF16 = ml_dtypes.bfloat16
F8E4 = ml_dtypes.float8_e4m3
F32 = mybir.dt.float32
BF = mybir.dt.bfloat16
FP8 = mybir.dt.float8e4
AF = mybir.ActivationFunctionType
ALU = mybir.AluOpType
DRM = mybir.MatmulPerfMode.DoubleRow
# conv scaling: c2p holds 8x the true conv1 output (fp8 range), w2 is
# stored 32x (fp8 range); the conv2 output activation folds 1/(8*32) and
# the 1/4096 mean-pool divisor into its scale.
SA = 1.0 / (4096.0 * 8.0 * 32.0)


def build_program(ncores=8, npairs=8, nsteps=128):
    nfr = 2 * npairs                 # frames per core
    T = ncores * nfr                 # total frames/steps available
    assert nsteps <= T
    nc = bacc.Bacc("TRN2", target_bir_lowering=False, debug=False,
                   num_devices=ncores)

    din = lambda n, s, d: nc.dram_tensor(n, s, d, kind="ExternalInput")
    frames = din("frames", [nfr, 3, 256, 256], F32)
    w1bd = din("w1bd", [54, 128], FP8)
    b1 = din("b1", [128, 1], F32)
    w2f8 = din("w2f8", [128, 5, 2, 128], FP8)
    b2s = din("b2s", [128, 1], F32)
    enc = din("enc", [14, 2048], F32)
    fceT = din("fceT", [128, 2, 16, 128], BF)
    fceb2 = din("fceb2", [128, 1], F32)
    fcw2T = din("fcw2T", [128, 7, 400], BF)
    fcw1T = din("fcw1T", [128, 400], BF)
    fcb = din("fcb", [100, 4], F32)
    AT = din("AT", [100, 4, 512], BF)
    A2T = din("A2T", [100, 4, 512], BF)
    baT = din("baT", [128, 4], F32)
    F3Tp = din("F3Tp", [128, 4, 4, 128], BF)
    act_in = din("act_in", [1, 400], F32)
    ident = din("ident", [128, 128], F32)
    identb = din("identb", [128, 128], BF)

    out = nc.dram_tensor("out", [nsteps, 400], F32, kind="ExternalOutput")
    feats_loc = nc.dram_tensor("feats_loc", [128, nfr], F32)
    feats_glob = nc.dram_tensor("feats_glob", [128 * ncores, nfr], F32,
                                addr_space="Shared")

    with tile.TileContext(nc) as tc:
        with tc.tile_pool(name="const", bufs=1) as cp:
            w1bd_t = cp.tile([54, 128], FP8)
            nc.sync.dma_start(out=w1bd_t[:], in_=w1bd[:])
            b1_t = cp.tile([128, 1], F32)
            nc.sync.dma_start(out=b1_t[:], in_=b1[:])
            w2f8_t = cp.tile([128, 5, 2, 128], FP8)
            nc.sync.dma_start(out=w2f8_t[:], in_=w2f8[:])
            b2s_t = cp.tile([128, 1], F32)
            nc.sync.dma_start(out=b2s_t[:], in_=b2s[:])
            ident_t = cp.tile([128, 128], F32)
            nc.sync.dma_start(out=ident_t[:], in_=ident[:])
            identb_t = cp.tile([128, 128], BF)
            nc.sync.dma_start(out=identb_t[:], in_=identb[:])

            # ---------------- conv stage ----------------
            with tc.tile_pool(name="cv", bufs=3) as cv, \
                 tc.tile_pool(name="cv1", bufs=2) as cv1, \
                 tc.tile_pool(name="ps1", bufs=2, space="PSUM") as ps1, \
                 tc.tile_pool(name="ps2", bufs=2, space="PSUM") as ps2:
                feats_sb = cv1.tile([128, nfr], F32)
                for j in range(npairs):
                    FR = cv.tile([128, 2, 3, 2, 128, 2], F32, tag="FR")
                    for f in range(2):
                        feng = nc.sync if f == 0 else nc.scalar
                        feng.dma_start(out=FR[:, f], in_=frames[2 * j + f].rearrange("c (p r) x -> p c (r x)", r=2))
                    # column-phase compaction (cast f32 -> fp8e4)
                    Q = {}
                    for f in range(2):
                        for rp in range(2):  # row parity
                            for cpar in range(2):  # col parity
                                q = cv.tile([128, 3, 130], FP8,
                                            tag=f"Q{f}{rp}{cpar}")
                                Q[(f, rp, cpar)] = q
                                j0 = cpar  # odd cols shift right by one
                                nc.vector.tensor_copy(
                                    q[:, :, j0:j0 + 128],
                                    FR[:, f, :, rp, :, cpar])
                                if cpar == 1:
                                    nc.vector.memset(q[:, :, 0:1], 0.0)
                    IC = cv.tile([54, 128, 128], FP8, tag="IC")
                    nc.vector.memset(IC[:, 0:1, :], 0.0)
                    qi = 0
                    for f in range(2):
                        for ic in range(3):
                            for ky in range(3):
                                for kx in range(3):
                                    t = 27 * f + 9 * ic + 3 * ky + kx
                                    rp = (ky + 1) % 2  # ky=1 -> even rows
                                    cpar = (kx + 1) % 2
                                    q = Q[(f, rp, cpar)]
                                    j0 = 0 if kx < 2 else 1
                                    if ky == 0:
                                        src = q[0:127, ic, j0:j0 + 128]
                                        dst = IC[t:t + 1, 1:128, :]
                                    else:
                                        src = q[0:128, ic, j0:j0 + 128]
                                        dst = IC[t:t + 1, 0:128, :]
                                    if j == 0:
                                        eng = (nc.sync, nc.gpsimd,
                                               nc.scalar)[qi % 3]
                                    else:
                                        eng = nc.gpsimd if qi % 2 \
                                            else nc.sync
                                    eng.dma_start(out=dst, in_=src)
                                    qi += 1
                    ICf = IC.rearrange("t a b -> t (a b)")
                    c2p = cv1.tile([128, 130, 130], FP8, tag="c2p")
                    nc.vector.memset(c2p[:, 0, :], 0.0)
                    nc.vector.memset(c2p[:, 129, :], 0.0)
                    nc.vector.memset(c2p[:, :, 0:1], 0.0)
                    nc.vector.memset(c2p[:, :, 129:130], 0.0)
                    for n in range(16):
                        pm = ps1.tile([128, 1024], F32, tag="pm")
                        for half in range(2):
                            c0 = 1024 * n + 512 * half
                            nc.tensor.matmul(pm[:, 512 * half:512 * half + 512],
                                             w1bd_t[:], ICf[:, c0:c0 + 512],
                                             start=True, stop=True)
                        dst = c2p[:, 1 + 8 * n:9 + 8 * n, 1:129]
                        src = pm.rearrange("p (a b) -> p a b", b=128)
                        if n % 2 == 0:
                            nc.scalar.activation(dst, src, AF.Relu,
                                                 bias=b1_t[:, 0:1])
                        else:
                            nc.vector.tensor_scalar(
                                out=dst, in0=src, scalar1=b1_t[:, 0:1],
                                scalar2=0.0, op0=ALU.add, op1=ALU.max)
                    # conv2 per frame half: 4 DoubleRow tap-pair matmuls +
                    # one single-tap matmul per 512-column chunk
                    facc = cv.tile([128, 2, 4], F32, tag="facc")
                    for f in range(2):
                        for h in range(4):
                          pc = ps2.tile([128, 1024], F32, tag="pc")
                          for half in range(2):
                            n2 = 2 * h + half
                            po = pc[:, 512 * half:512 * half + 512]
                            base = 16 * n2
                            c2f = c2p[64 * f:64 * f + 64]
                            for s in range(3):  # taps (s,0)+(s,1)
                                rhs = c2f[:, s + base:s + base + 16:2,
                                          0:128].rearrange(
                                    "p i (c two) -> p two i c", two=2)
                                nc.tensor.matmul(po, w2f8_t[64 * f:64 * f + 64, s],
                                                 rhs, start=(s == 0),
                                                 stop=False, perf_mode=DRM)
                            # taps (0,2)+(1,2) paired on row parity
                            rhs = c2f[:, base:base + 16,
                                      2:130:2].rearrange(
                                "p (i two) c -> p two i c", two=2)
                            nc.tensor.matmul(po, w2f8_t[64 * f:64 * f + 64, 3], rhs,
                                             start=False, stop=False,
                                             perf_mode=DRM)
                            # tap (2,2) single
                            rhs = c2f[:, 2 + base:2 + base + 16:2, 2:130:2]
                            nc.tensor.matmul(po, w2f8_t[64 * f:64 * f + 64, 4, 0], rhs,
                                             start=False, stop=True)
                          junk = cv.tile([128, 1024], FP8, tag="junk")
                          if h % 2 == 0:
                              nc.scalar.activation(
                                  junk[:], pc[:], AF.Relu,
                                  bias=b2s_t[:, 0:1], scale=SA,
                                  accum_out=facc[:, f, h:h + 1])
                          else:
                              nc.vector.tensor_scalar(
                                  out=junk[:], in0=pc[:],
                                  scalar1=SA, scalar2=0.0,
                                  op0=ALU.mult, op1=ALU.max,
                                  accum_out=facc[:, f, h:h + 1])
                    for f in range(2):
                        nc.vector.reduce_sum(
                            out=feats_sb[:, 2 * j + f:2 * j + f + 1],
                            in_=facc[:, f, :], axis=mybir.AxisListType.X)
                nc.sync.dma_start(out=feats_loc[:], in_=feats_sb[:])
                nc.gpsimd.collective_compute(
                    "AllGather", ALU.bypass,
                    replica_groups=[list(range(ncores))],
                    ins=[feats_loc[:]], outs=[feats_glob[:]])

            # ---------------- ev branch + Lpre ----------------
            sc1 = tc.tile_pool(name="sc1", bufs=1)
            scp = sc1.__enter__()
            Lpre = scp.tile([100, 4, nsteps], F32)
            Lh = scp.tile([100, 4, nsteps], BF)
            Ll = scp.tile([100, 4, nsteps], BF)
            U = scp.tile([100, 4, nsteps], BF)
            ones100 = scp.tile([100, 128], BF)
            nc.vector.memset(ones100[:], 1.0)

            with tc.tile_pool(name="ev", bufs=2) as evp, \
                 tc.tile_pool(name="pse", bufs=2, space="PSUM") as pse, \
                 tc.tile_pool(name="pse1", bufs=1, space="PSUM") as pse1:
                # gather feats -> [128, T] bf16, t = ncores*jloc + c
                fa = evp.tile([128, ncores, nfr], F32)
                nc.sync.dma_start(out=fa[:], in_=feats_glob[:].rearrange(
                    "(c p) j -> p c j", p=128))
                fb = scp.tile([128, T], BF)
                fbv = fb.rearrange("p (j c) -> p c j", c=ncores)
                nc.vector.tensor_copy(fbv, fa[:])

                # encoded_video branch
                enc_sb = evp.tile([14, 2048], F32)
                nc.sync.dma_start(out=enc_sb[:], in_=enc[:])
                fceT_t = evp.tile([128, 2, 16, 128], BF, tag="fceT")
                nc.sync.dma_start(out=fceT_t[:], in_=fceT[:])
                encT = evp.tile([128, 16, 14], BF)
                for k in range(16):
                    pt = pse.tile([128, 14], F32, tag="pt")
                    nc.tensor.transpose(pt[:], enc_sb[:, 128 * k:128 * k + 128],
                                        ident_t[0:14, 0:14])
                    nc.vector.tensor_copy(encT[:, k, :], pt[:])
                pev = pse1.tile([128, 7], F32)
                for k in range(16):
                    for par in range(2):
                        nc.tensor.matmul(
                            pev[:], fceT_t[:, par, k, :],
                            encT[:, k, par:14:2],
                            start=(k == 0 and par == 0),
                            stop=(k == 15 and par == 1))
                fceb2_t = evp.tile([128, 1], F32)
                nc.sync.dma_start(out=fceb2_t[:], in_=fceb2[:])
                ev_sb = evp.tile([128, 7], BF)
                nc.scalar.activation(ev_sb[:], pev[:], AF.Relu,
                                     bias=fceb2_t[:, 0:1])
                fcw2T_t = evp.tile([128, 7, 400], BF, tag="fcw2T")
                nc.sync.dma_start(out=fcw2T_t[:], in_=fcw2T[:])
                pevl = pse1.tile([100, 4], F32)
                for m in range(4):
                    for k in range(7):
                        nc.tensor.matmul(
                            pevl[:, m:m + 1],
                            fcw2T_t[:, k, 100 * m:100 * m + 100],
                            ev_sb[:, k:k + 1], start=(k == 0), stop=(k == 6))
                fcb_t = evp.tile([100, 4], F32)
                nc.sync.dma_start(out=fcb_t[:], in_=fcb[:])
                bconst = evp.tile([100, 4], F32)
                nc.vector.tensor_add(bconst[:], pevl[:], fcb_t[:])
                fcw1T_t = evp.tile([128, 400], BF)
                nc.sync.dma_start(out=fcw1T_t[:], in_=fcw1T[:])
                for m in range(4):
                    plp = pse.tile([100, nsteps], F32, tag="plp")
                    nc.tensor.matmul(plp[:],
                                     fcw1T_t[:, 100 * m:100 * m + 100],
                                     fb[:, 0:nsteps], start=True, stop=True)
                    nc.vector.tensor_scalar_add(Lpre[:, m, :], plp[:],
                                                bconst[:, m:m + 1])
                    nc.vector.tensor_copy(Lh[0:100, m, :], Lpre[0:100, m, :])
                    nc.vector.tensor_sub(Ll[0:100, m, :], Lpre[0:100, m, :],
                                         Lh[0:100, m, :])

            # ---------------- scan ----------------
            AT_t = scp.tile([100, 4, 512], BF)
            nc.sync.dma_start(out=AT_t[:], in_=AT[:])
            A2T_t = scp.tile([100, 4, 512], BF)
            nc.sync.dma_start(out=A2T_t[:], in_=A2T[:])
            baT_t = scp.tile([128, 4], F32)
            nc.sync.dma_start(out=baT_t[:], in_=baT[:])
            F3Tp_t = scp.tile([128, 4, 4, 128], BF)
            nc.sync.dma_start(out=F3Tp_t[:], in_=F3Tp[:])

            with tc.tile_pool(name="sstep", bufs=3) as ssp, \
                 tc.tile_pool(name="psA", bufs=2, space="PSUM") as psA, \
                 tc.tile_pool(name="psL", bufs=2, space="PSUM") as psL, \
                 tc.tile_pool(name="psS", bufs=2, space="PSUM") as psS:
                # step 0 input: transpose action row into [100, 4]
                act_row = ssp.tile([1, 400], F32, tag="actrow")
                nc.sync.dma_start(out=act_row[:], in_=act_in[:])
                u0 = ssp.tile([100, 4], BF, tag="u0")
                for m in range(4):
                    pa0 = psS.tile([128, 4], F32, tag="S")
                    nc.tensor.transpose(pa0[0:100, 0:1],
                                        act_row[0:1, 100 * m:100 * m + 100],
                                        ident_t[0:1, 0:1])
                    nc.vector.tensor_copy(u0[:, m:m + 1], pa0[0:100, 0:1])

                for t in range(nsteps):
                    Wmat = AT_t if t == 0 else A2T_t
                    ucur = (lambda k: u0[:, k:k + 1]) if t == 0 else \
                        (lambda k, tt=t - 1: U[0:100, k, tt:tt + 1])
                    # normalizer: S = colsum(u_{t-1}) broadcast to 128 rows
                    if t > 0:
                        S_ps = psS.tile([128, 4], F32, tag="S")
                        nc.tensor.matmul(S_ps[:], ones100[:],
                                         U[0:100, :, t - 1],
                                         start=True, stop=True)
                    a_ps = psA.tile([128, 4], F32, tag="a_ps")
                    for m in range(4):
                        for k in range(4):
                            nc.tensor.matmul(
                                a_ps[:, m:m + 1],
                                Wmat[:, k, 128 * m:128 * m + 128],
                                ucur(k), start=(k == 0), stop=(k == 3))
                    w_sb = ssp.tile([128, 4], BF, tag="w_sb")
                    if t == 0:
                        for m in range(4):
                            nc.vector.tensor_scalar(
                                out=w_sb[:, m:m + 1], in0=a_ps[:, m:m + 1],
                                scalar1=baT_t[:, m:m + 1], scalar2=0.0,
                                op0=ALU.add, op1=ALU.max)
                    else:
                        s1 = ssp.tile([128, 1], F32, tag="s1")
                        nc.vector.reduce_sum(out=s1[:], in_=S_ps[:],
                                             axis=mybir.AxisListType.X)
                        r_t = ssp.tile([128, 1], F32, tag="r_t")
                        nc.vector.reciprocal(r_t[:], s1[:])
                        nc.vector.tensor_scalar(
                            out=w_sb[:], in0=a_ps[:],
                            scalar1=r_t[:, 0:1], scalar2=0.0,
                            op0=ALU.mult, op1=ALU.max)
                    # logits = F3.w + Lpre (hi+lo), all in one PSUM group
                    l_ps = psL.tile([128, 4], F32, tag="l_ps")
                    nc.tensor.matmul(l_ps[:], identb_t[0:100, :],
                                     Lh[0:100, :, t], start=True, stop=False)
                    nc.tensor.matmul(l_ps[:], identb_t[0:100, :],
                                     Ll[0:100, :, t], start=False, stop=False)
                    for m in range(4):
                        for k in range(4):
                            nc.tensor.matmul(
                                l_ps[:, m:m + 1],
                                F3Tp_t[:, k, m, :],
                                w_sb[:, k:k + 1], start=False,
                                stop=(k == 3))
                    nc.scalar.activation(U[0:100, :, t], l_ps[0:100, :],
                                         AF.Exp)

                # final: transpose U to [t, c] layout and normalize rows
                OT = ssp.tile([128, 400], F32, tag="OT")
                for m in range(4):
                    tp_ps = psA.tile([128, 100], BF, tag="tp_ps")
                    nc.tensor.transpose(tp_ps[0:nsteps, :],
                                        U[0:100, m, 0:nsteps],
                                        identb_t[0:100, 0:100])
                    nc.vector.tensor_copy(
                        OT[0:nsteps, 100 * m:100 * m + 100],
                        tp_ps[0:nsteps, :])
                rowsum = ssp.tile([128, 1], F32, tag="rowsum")
                nc.vector.reduce_sum(out=rowsum[0:nsteps],
                                     in_=OT[0:nsteps, :],
                                     axis=mybir.AxisListType.X)
                rinv = ssp.tile([128, 1], F32, tag="rinv")
                nc.vector.reciprocal(rinv[0:nsteps], rowsum[0:nsteps])
                OTn = ssp.tile([128, 400], F32, tag="OTn")
                nc.vector.tensor_scalar_mul(OTn[0:nsteps, :],
                                            OT[0:nsteps, :],
                                            rinv[0:nsteps, 0:1])
                nc.sync.dma_start(out=out[:], in_=OTn[0:nsteps, :])
            sc1.__exit__(None, None, None)

    nc.compile()
    return nc


def prep_weights(inputs, ncores=8):
    """Host-side numpy prep of all weight layouts. Returns dict of arrays
    shared by all cores (frames excluded)."""
    f32 = np.float32
    conv1_w = np.asarray(inputs["conv1_w"], f32)
    conv2_w = np.asarray(inputs["conv2_w"], f32)
    w1bd = np.zeros((54, 128), f32)
    for f in range(2):
        for ic in range(3):
            for ky in range(3):
                for kx in range(3):
                    t = 27 * f + 9 * ic + 3 * ky + kx
                    w1bd[t, 64 * f:64 * f + 64] = conv1_w[:, ic, ky, kx]
    w1bd *= 8.0  # c2p holds 8x conv1 output for fp8 range
    b1 = np.tile(np.asarray(inputs["conv1_b"], f32) * 8.0, 2).reshape(128, 1)
    # conv2 weights as DoubleRow tap-pair tiles, 32x scaled for fp8 range
    w2t = conv2_w.transpose(1, 2, 3, 0)           # [ic, ky, kx, oc]
    w2f8 = np.zeros((64, 5, 2, 128), f32)
    for s in range(3):
        w2f8[:, s, 0, :] = w2t[:, s, 0, :]
        w2f8[:, s, 1, :] = w2t[:, s, 1, :]
    w2f8[:, 3, 0, :] = w2t[:, 0, 2, :]
    w2f8[:, 3, 1, :] = w2t[:, 1, 2, :]
    w2f8[:, 4, 0, :] = w2t[:, 2, 2, :]
    w2f8 *= 32.0
    w2f8 = np.concatenate([w2f8, w2f8], axis=0)  # both partition halves
    b2s = (np.asarray(inputs["conv2_b"], f32) / 4096.0).reshape(128, 1)

    enc = np.asarray(inputs["encoded_video"], f32).reshape(14, 2048)
    fce_w = np.asarray(inputs["fce_w"], f32)      # [64, 2048]
    fceT = np.zeros((128, 2, 16, 128), f32)
    fceTf = fce_w.T.reshape(16, 128, 64)      # [k, p, o]
    fceT[:, 0, :, 0:64] = fceTf.transpose(1, 0, 2)
    fceT[:, 1, :, 64:128] = fceTf.transpose(1, 0, 2)
    fceb2 = np.tile(np.asarray(inputs["fce_b"], f32), 2).reshape(128, 1)

    fc_w = np.asarray(inputs["fc_w"], f32)        # [400, 1536]
    fc_b = np.asarray(inputs["fc_b"], f32)
    fcw1T = fc_w[:, 0:128].T.copy()               # [128, 400]
    fcw2T = fc_w[:, 128:1024].T.reshape(7, 128, 400).transpose(1, 0, 2).copy()
    F3t = fc_w[:, 1024:1536].T                    # [512, 400]
    F3Tp = np.zeros((128, 4, 4, 128), f32)
    for k in range(4):
        for m in range(4):
            F3Tp[:, k, m, 0:100] = F3t[128 * k:128 * k + 128,
                                       100 * m:100 * m + 100]
    fcb = fc_b.reshape(4, 100).T.copy()           # [100, 4]

    fca_w = np.asarray(inputs["fca_w"], f32)      # [512, 400]
    fca_b = np.asarray(inputs["fca_b"], f32)
    ATm = fca_w.T.reshape(4, 100, 512).transpose(1, 0, 2).copy()
    A2 = fca_w + fca_b[:, None]
    A2Tm = A2.T.reshape(4, 100, 512).transpose(1, 0, 2).copy()
    baT = fca_b.reshape(4, 128).T.copy()          # [128, 4]

    action = np.asarray(inputs["action"], f32).reshape(1, 400)

    bf = lambda x: np.ascontiguousarray(x).astype(BF16)
    f8 = lambda x: np.ascontiguousarray(x).astype(F8E4)
    return {
        "w1bd": f8(w1bd), "b1": b1, "w2f8": f8(w2f8), "b2s": b2s,
        "enc": enc, "fceT": bf(fceT), "fceb2": fceb2,
        "fcw2T": bf(fcw2T), "fcw1T": bf(fcw1T), "fcb": fcb,
        "AT": bf(ATm), "A2T": bf(A2Tm), "baT": baT, "F3Tp": bf(F3Tp),
        "act_in": action,
        "ident": np.eye(128, dtype=f32),
        "identb": np.eye(128, dtype=f32).astype(BF16),
    }


_CACHE = {}


def _run(inputs, trace=False):
    from concourse.bass_utils import run_bass_kernel_spmd
    ncores = 8
    if "nc" not in _CACHE:
        _CACHE["nc"] = build_program(ncores=ncores, npairs=8, nsteps=128)
    nc = _CACHE["nc"]
    common = prep_weights(inputs, ncores)
    vf = np.asarray(inputs["video_frame"], np.float32)[0]
    in_maps = []
    for c in range(ncores):
        m = dict(common)
        m["frames"] = np.ascontiguousarray(vf[c::ncores])
        in_maps.append(m)
    return run_bass_kernel_spmd(nc, in_maps, core_ids=list(range(ncores)),
                                trace=trace)


def kernel(**inputs):
    res = _run(inputs, trace=False)
    return res.results[0]["out"].reshape(1, 128, 400).astype(np.float32)

